# revision 15
# baseline (speedup 1.0000x reference)
"""2-layer GCN on 8 Trainium2 NeuronCores (Bass/Tile).

Node (dst) sharding across 8 cores. Host builds a static, core-uniform
gather/scatter schedule: edges bucketed by src%4 (table row = packed_row//4,
int16-safe), sorted by dst, tiled into 128-slot scatter tiles per
(bucket, dst-region-of-128); tile counts padded to the max across cores so
one SPMD program serves all cores. Per layer: chunked dma_gather (SWDGE,
64B payload / 256B stride) fetches y[src] edge-major; PE one-hot matmuls
segment-sum into persistent PSUM region banks; AllGather (DRAM bounce)
shares per-shard y tables between layers.

Math: z = x@W1; y1 = dinv*z; agg = sum_edges y[src] (per dst);
h = relu(dinv*agg1 + dinv*y1 + b1); y2 = dinv*h;
out = log_softmax((dinv*agg2 + dinv*y2)@W2 + b2).
"""
import sys

sys.path.insert(0, "/opt/trn_rl_repo")
import numpy as np

import concourse.bass as bass
import concourse.bacc as bacc
import concourse.tile as tile
import concourse.mybir as mybir
from concourse.bass_utils import run_bass_kernel_spmd

N_NODES = 100000
F_IN, F_HID, F_OUT = 128, 16, 20
NC = 8
SHARD = N_NODES // NC           # 12500
NREG = (SHARD + 127) // 128     # 98
PSHARD = NREG * 128             # 12544 (padded shard rows in table)
NTAB = NC * PSHARD              # 100352 table rows
NBUCKET = 4                     # src row % 4
CHUNK = 1024
TPC = CHUNK // 128              # tiles per chunk = 8
NQUEUES = 4

f32 = mybir.dt.float32
f32r = mybir.dt.float32r
bf16 = mybir.dt.bfloat16
i32 = mybir.dt.int32
i16 = mybir.dt.int16


def _row_of(node):
    return (node // SHARD) * PSHARD + (node % SHARD)


def _preprocess(edge_index):
    src = edge_index[0].astype(np.int64)
    dst = edge_index[1].astype(np.int64)
    deg = np.bincount(dst, minlength=N_NODES).astype(np.float64) + 1.0
    dinv_all = (1.0 / np.sqrt(deg)).astype(np.float32)
    srow = _row_of(src)

    per_core_ed = []
    counts = np.zeros((NC, NBUCKET, NREG), np.int64)
    for c in range(NC):
        lo = c * SHARD
        m = (dst >= lo) & (dst < lo + SHARD)
        s, d = srow[m], dst[m] - lo
        b = s % 4
        order = np.lexsort((d, b))
        s, d, b = s[order], d[order], b[order]
        reg = d // 128
        np.add.at(counts[c], (b, reg), 1)
        per_core_ed.append((s, d, b, reg))

    # uniform tile counts per (bucket, region)
    tcnt = np.maximum(np.ceil(counts / 128.0).astype(np.int64).max(axis=0), 1)
    ntiles = int(tcnt.sum())

    # static schedule (same for all cores): per tile -> (bucket, region);
    # pad each bucket to a multiple of TPC with dummy (b, 0) tiles so no
    # chunk crosses a bucket boundary.
    sched = []
    for b in range(NBUCKET):
        blist = []
        for r in range(NREG):
            blist.extend([(b, r)] * int(tcnt[b, r]))
        while len(blist) % TPC:
            blist.append((b, 0))
        sched.extend(blist)
    ntiles_pad = len(sched)
    nchunks = ntiles_pad // TPC

    first, last = {}, {}
    for t, (b, r) in enumerate(sched):
        if r not in first:
            first[r] = t
        last[r] = t

    chunk_bucket = [sched[ci * TPC][0] for ci in range(nchunks)]
    # tiles within one chunk must share the bucket (gather src sub-table).
    for ci in range(nchunks):
        bs = {sched[ci * TPC + k][0] for k in range(TPC)}
        assert len(bs) == 1, f"chunk {ci} mixes buckets {bs}"

    meta = dict(ntiles=ntiles_pad, nchunks=nchunks, sched=sched,
                first=first, last=last, chunk_bucket=chunk_bucket)

    # per-core data arrays following the schedule
    per_core = []
    for c in range(NC):
        s, d, b, reg = per_core_ed[c]
        idx = np.zeros((ntiles_pad, 128), np.int16)
        dcol = np.full((ntiles_pad, 128), -1.0, np.float32)
        # edges of (b, r) are contiguous in (s, d) by construction
        starts = {}
        pos = 0
        for bb in range(NBUCKET):
            for r in range(NREG):
                starts[(bb, r)] = pos
                pos += int(counts[c, bb, r])
        fill = {}
        for t, (bb, r) in enumerate(sched):
            fill.setdefault((bb, r), []).append(t)
        for (bb, r), tlist in fill.items():
            p0 = starts.get((bb, r))
            if p0 is None:
                continue
            cnt = int(counts[c, bb, r]) if (bb, r) in starts else 0
            for k, t in enumerate(tlist):
                a, z_ = p0 + k * 128, p0 + min((k + 1) * 128, cnt)
                n = z_ - a
                if n <= 0:
                    continue
                idx[t, :n] = (s[a:z_] // 4).astype(np.int16)
                dcol[t, :n] = (d[a:z_] - r * 128).astype(np.float32)
        # wrap idx into [128, ntiles_pad*8] int16 (16-partition wrap, x8)
        iw = idx.reshape(ntiles_pad * 8, 16).T     # [16, ntiles*8]
        iw = np.tile(iw, (8, 1)).astype(np.int16)  # [128, ntiles*8]
        dct = dcol.T.copy()                        # [128, ntiles_pad]

        lo = c * SHARD
        dv = dinv_all[lo:lo + SHARD]
        dv = np.concatenate([dv, np.zeros(PSHARD - SHARD, np.float32)])
        dv = dv.reshape(NREG, 128).T.copy()        # [128, NREG]
        per_core.append(dict(idx=iw, dstcol=dct, dinv_col=dv))
    return dinv_all, meta, per_core


def _dma_gather_raw(gpsimd, out_ap, in_ap, idxs_ap, num_idxs, elem_size,
                    elem_step, queue_num=0):
    stride_bytes = elem_step * mybir.dt.size(in_ap.dtype)
    assert stride_bytes % 256 == 0
    _in_ap = gpsimd.lower_ap_dma(in_ap, for_custom_bir_dma=True)
    _idxs_ap = gpsimd.lower_ap(idxs_ap)
    _out_ap = gpsimd.lower_ap(out_ap)
    return gpsimd.add_instruction(
        mybir.InstDMAGatherAnt(
            name=gpsimd.bass.get_next_instruction_name(),
            ins=[*_in_ap, _idxs_ap,
                 gpsimd.lower_val_access(gpsimd.to_reg(num_idxs))],
            outs=[_out_ap],
            transpose=False, num_idxs=num_idxs, elem_size=elem_size,
            stride_bytes_256=stride_bytes // 256, gen_mode=0,
            single_packet=True, queue_num=queue_num,
            sbuf_tokens_per_rank=0, sbuf_free_dim_per_rank=0,
            sbuf_free_dim_pad_per_rank=0, sbuf_byte_offset=0,
        ))


def _build(meta):
    nchunks = meta["nchunks"]
    ntiles = meta["ntiles"]
    sched = meta["sched"]
    first, last = meta["first"], meta["last"]
    chunk_bucket = meta["chunk_bucket"]

    nc = bacc.Bacc("TRN2", target_bir_lowering=False, debug=False,
                   num_devices=NC, num_swdge_queues=NQUEUES)

    xh_d = nc.dram_tensor("xTh", [F_IN, SHARD], bf16, kind="ExternalInput")
    xl_d = nc.dram_tensor("xTl", [F_IN, SHARD], bf16, kind="ExternalInput")
    w1h_d = nc.dram_tensor("W1h", [F_IN, F_HID], bf16, kind="ExternalInput")
    w1l_d = nc.dram_tensor("W1l", [F_IN, F_HID], bf16, kind="ExternalInput")
    w2h_d = nc.dram_tensor("W2h", [F_HID, F_OUT], bf16, kind="ExternalInput")
    w2l_d = nc.dram_tensor("W2l", [F_HID, F_OUT], bf16, kind="ExternalInput")
    b1_d = nc.dram_tensor("b1t", [128, F_HID], f32, kind="ExternalInput")
    b2_d = nc.dram_tensor("b2t", [128, F_OUT], f32, kind="ExternalInput")
    dinv_d = nc.dram_tensor("dinv_col", [128, NREG], f32, kind="ExternalInput")
    iota_d = nc.dram_tensor("iota8", [128, TPC, 128], f32, kind="ExternalInput")
    ident_d = nc.dram_tensor("ident", [128, 128], f32, kind="ExternalInput")
    idx_d = nc.dram_tensor("idx", [128, ntiles * 8], i16, kind="ExternalInput")
    dc_d = nc.dram_tensor("dstcol", [128, ntiles], f32, kind="ExternalInput")
    out_d = nc.dram_tensor("out", [128, NREG * F_OUT], f32,
                           kind="ExternalOutput")


    ybounce = nc.dram_tensor("ybounce", [PSHARD, F_HID], f32)
    ytab1 = nc.dram_tensor("ytab1", [NTAB, F_HID], f32, addr_space="Shared")
    hbounce = nc.dram_tensor("hbounce", [PSHARD, F_HID], f32)
    ytab2 = nc.dram_tensor("ytab2", [NTAB, F_HID], f32, addr_space="Shared")

    with tile.TileContext(nc) as tc:
        with tc.tile_pool(name="const", bufs=1) as cpool, \
             tc.tile_pool(name="work", bufs=3) as wpool, \
             tc.tile_pool(name="gath", bufs=6) as gpool, \
             tc.tile_pool(name="big", bufs=1) as apool, \
             tc.tile_pool(name="ps", bufs=2, space="PSUM") as pspool, \
             tc.tile_pool(name="psacc", bufs=1, space="PSUM") as papool:

            w1h = cpool.tile([F_IN, F_HID], bf16)
            w1l = cpool.tile([F_IN, F_HID], bf16)
            w2h = cpool.tile([F_HID, F_OUT], bf16)
            w2l = cpool.tile([F_HID, F_OUT], bf16)
            b1t = cpool.tile([128, F_HID], f32)
            b2t = cpool.tile([128, F_OUT], f32)
            dinvc = cpool.tile([128, NREG], f32)
            iota8 = cpool.tile([128, TPC, 128], f32)
            ident = cpool.tile([128, 128], f32)
            for t_, d_ in ((w1h, w1h_d), (w1l, w1l_d), (w2h, w2h_d),
                           (w2l, w2l_d), (b1t, b1_d), (b2t, b2_d),
                           (dinvc, dinv_d), (iota8, iota_d), (ident, ident_d)):
                nc.sync.dma_start(out=t_[:], in_=d_[:])

            # phase A: z = x @ W1, dst-major
            xTh = apool.tile([F_IN, SHARD], bf16)
            xTl = apool.tile([F_IN, SHARD], bf16)
            nc.sync.dma_start(out=xTh[:], in_=xh_d[:])
            nc.sync.dma_start(out=xTl[:], in_=xl_d[:])
            z = apool.tile([128, NREG, F_HID], f32)
            for r in range(NREG):
                n0, n1 = r * 128, min(r * 128 + 128, SHARD)
                zp = pspool.tile([128, 128], f32, tag="pp")
                nc.tensor.matmul(out=zp[:n1 - n0, :F_HID],
                                 lhsT=xTh[:, n0:n1], rhs=w1h[:],
                                 start=True, stop=False, skip_group_check=True)
                nc.tensor.matmul(out=zp[:n1 - n0, :F_HID],
                                 lhsT=xTh[:, n0:n1], rhs=w1l[:],
                                 start=False, stop=False, skip_group_check=True)
                nc.tensor.matmul(out=zp[:n1 - n0, :F_HID],
                                 lhsT=xTl[:, n0:n1], rhs=w1h[:],
                                 start=False, stop=True, skip_group_check=True)
                if n1 - n0 < 128:
                    nc.vector.memset(z[:, r, :], 0.0)
                nc.vector.tensor_copy(out=z[:n1 - n0, r, :],
                                      in_=zp[:n1 - n0, :F_HID])

            dinv_b = dinvc[:].rearrange("p (r o) -> p r o", o=1)

            def aggregate(ytab, acc):
                nbank = (NREG + 31) // 32
                banks = []
                for i in range(nbank):
                    bk = papool.tile([128, 32 * F_HID], f32, tag=f"bk{i}",
                                     name=f"bank{i}")
                    banks.append(bk)
                for ci in range(nchunks):
                    b = chunk_bucket[ci]
                    gt = gpool.tile([128, TPC, F_HID], f32, tag="g")
                    ix = gpool.tile([128, CHUNK // 16], i16, tag="ix")
                    dc = gpool.tile([128, TPC], f32, tag="dc")
                    sel = gpool.tile([128, TPC, 128], bf16, tag="sel")
                    gth = gpool.tile([128, TPC, F_HID], bf16, tag="gth")
                    gthf = gpool.tile([128, TPC, F_HID], f32, tag="gthf")
                    gtl = gpool.tile([128, TPC, F_HID], bf16, tag="gtl")
                    nc.sync.dma_start(
                        out=ix[:],
                        in_=idx_d[:, ci * (CHUNK // 16):(ci + 1) * (CHUNK // 16)])
                    nc.sync.dma_start(out=dc[:],
                                      in_=dc_d[:, ci * TPC:(ci + 1) * TPC])
                    in_ap = bass.AP(ytab, b * F_HID,
                                    [[4 * F_HID, NTAB // 4], [1, F_HID]])
                    _dma_gather_raw(nc.gpsimd, gt[:], in_ap, ix[:], CHUNK,
                                    F_HID, 4 * F_HID, queue_num=ci % NQUEUES)
                    nc.vector.tensor_tensor(
                        out=sel[:],
                        in0=dc[:].rearrange("p (t o) -> p t o", o=1).to_broadcast(
                            [128, TPC, 128]),
                        in1=iota8[:], op=mybir.AluOpType.is_equal)
                    nc.vector.tensor_copy(out=gth[:], in_=gt[:])
                    nc.vector.tensor_copy(out=gthf[:], in_=gth[:])
                    nc.vector.tensor_tensor(out=gtl[:], in0=gt[:], in1=gthf[:],
                                            op=mybir.AluOpType.subtract)
                    for tt in range(TPC):
                        t = ci * TPC + tt
                        _, reg = sched[t]
                        bank = banks[reg // 32]
                        off = (reg % 32) * F_HID
                        nc.tensor.matmul(
                            out=bank[:, off:off + F_HID],
                            lhsT=sel[:, tt, :], rhs=gth[:, tt, :],
                            start=(first[reg] == t), stop=False,
                            skip_group_check=True)
                        nc.tensor.matmul(
                            out=bank[:, off:off + F_HID],
                            lhsT=sel[:, tt, :], rhs=gtl[:, tt, :],
                            start=False, stop=(last[reg] == t),
                            skip_group_check=True)
                for bi, bank in enumerate(banks):
                    r0, r1 = bi * 32, min(bi * 32 + 32, NREG)
                    nc.vector.tensor_copy(
                        out=acc[:, r0:r1, :].rearrange("p r f -> p (r f)"),
                        in_=bank[:, :(r1 - r0) * F_HID])

            acc = apool.tile([128, NREG, F_HID], f32)

            # layer 1
            y1 = apool.tile([128, NREG, F_HID], f32)
            nc.vector.tensor_tensor(out=y1[:], in0=z[:],
                                    in1=dinv_b.to_broadcast([128, NREG, F_HID]),
                                    op=mybir.AluOpType.mult)
            nc.sync.dma_start(
                out=ybounce[:].rearrange("(r p) f -> p r f", p=128), in_=y1[:])
            nc.gpsimd.collective_compute(
                "AllGather", mybir.AluOpType.bypass,
                replica_groups=[list(range(NC))],
                ins=[ybounce[:]], outs=[ytab1[:]])
            aggregate(ytab1, acc)
            h = apool.tile([128, NREG, F_HID], f32)
            ta = apool.tile([128, NREG, F_HID], f32)
            tb = apool.tile([128, NREG, F_HID], f32)
            nc.vector.tensor_tensor(out=ta[:], in0=acc[:], in1=y1[:],
                                    op=mybir.AluOpType.add)
            nc.vector.tensor_tensor(out=tb[:], in0=ta[:],
                                    in1=dinv_b.to_broadcast([128, NREG, F_HID]),
                                    op=mybir.AluOpType.mult)
            nc.vector.tensor_tensor(
                out=ta[:], in0=tb[:],
                in1=b1t[:].rearrange("p (o f) -> p o f", o=1).to_broadcast(
                    [128, NREG, F_HID]),
                op=mybir.AluOpType.add)
            nc.scalar.activation(out=h[:], in_=ta[:],
                                 func=mybir.ActivationFunctionType.Relu)

            # layer 2
            y2 = y1
            nc.vector.tensor_tensor(out=y2[:], in0=h[:],
                                    in1=dinv_b.to_broadcast([128, NREG, F_HID]),
                                    op=mybir.AluOpType.mult)
            nc.sync.dma_start(
                out=hbounce[:].rearrange("(r p) f -> p r f", p=128), in_=y2[:])
            nc.gpsimd.collective_compute(
                "AllGather", mybir.AluOpType.bypass,
                replica_groups=[list(range(NC))],
                ins=[hbounce[:]], outs=[ytab2[:]])
            acc2 = apool.tile([128, NREG, F_HID], f32)
            aggregate(ytab2, acc2)
            tc2 = apool.tile([128, NREG, F_HID], f32)
            nc.vector.tensor_tensor(out=tc2[:], in0=acc2[:], in1=y2[:],
                                    op=mybir.AluOpType.add)
            nc.vector.tensor_tensor(out=acc2[:], in0=tc2[:],
                                    in1=dinv_b.to_broadcast([128, NREG, F_HID]),
                                    op=mybir.AluOpType.mult)

            out_sb = apool.tile([128, NREG, F_OUT], f32)
            for r in range(NREG):
                fp = pspool.tile([128, 128], f32, tag="pp")
                nc.tensor.transpose(out=fp[:F_HID, :], in_=acc2[:, r, :],
                                    identity=ident[:])
                fts = wpool.tile([F_HID, 128], f32, tag="ft")
                nc.vector.tensor_copy(out=fts[:], in_=fp[:F_HID, :])
                fh = wpool.tile([F_HID, 128], bf16, tag="fh")
                fhf = wpool.tile([F_HID, 128], f32, tag="fhf")
                fl = wpool.tile([F_HID, 128], bf16, tag="fl")
                nc.vector.tensor_copy(out=fh[:], in_=fts[:])
                nc.vector.tensor_copy(out=fhf[:], in_=fh[:])
                nc.vector.tensor_tensor(out=fl[:], in0=fts[:], in1=fhf[:],
                                        op=mybir.AluOpType.subtract)
                op = pspool.tile([128, 128], f32, tag="pp2")
                nc.tensor.matmul(out=op[:, :F_OUT], lhsT=fh[:], rhs=w2h[:],
                                 start=True, stop=False, skip_group_check=True)
                nc.tensor.matmul(out=op[:, :F_OUT], lhsT=fh[:], rhs=w2l[:],
                                 start=False, stop=False, skip_group_check=True)
                nc.tensor.matmul(out=op[:, :F_OUT], lhsT=fl[:], rhs=w2h[:],
                                 start=False, stop=True, skip_group_check=True)
                o = wpool.tile([128, F_OUT], f32, tag="o")
                nc.vector.tensor_tensor(out=o[:], in0=op[:, :F_OUT], in1=b2t[:],
                                        op=mybir.AluOpType.add)
                mx = wpool.tile([128, 1], f32, tag="mx")
                nc.vector.tensor_reduce(out=mx[:], in_=o[:],
                                        axis=mybir.AxisListType.X,
                                        op=mybir.AluOpType.max)
                nmx = wpool.tile([128, 1], f32, tag="nmx")
                nc.vector.tensor_scalar_mul(out=nmx[:], in0=mx[:], scalar1=-1.0)
                ex = wpool.tile([128, F_OUT], f32, tag="ex")
                sm = wpool.tile([128, 1], f32, tag="sm")
                nc.scalar.activation(out=ex[:], in_=o[:],
                                     func=mybir.ActivationFunctionType.Exp,
                                     bias=nmx[:, 0:1], accum_out=sm[:])
                ls = wpool.tile([128, 1], f32, tag="ls")
                nc.scalar.activation(out=ls[:], in_=sm[:],
                                     func=mybir.ActivationFunctionType.Ln)
                tot = wpool.tile([128, 1], f32, tag="tot")
                nc.vector.tensor_tensor(out=tot[:], in0=mx[:], in1=ls[:],
                                        op=mybir.AluOpType.add)
                nc.vector.tensor_tensor(
                    out=out_sb[:, r, :], in0=o[:],
                    in1=tot[:].to_broadcast([128, F_OUT]),
                    op=mybir.AluOpType.subtract)
            nc.sync.dma_start(out=out_d[:],
                              in_=out_sb[:].rearrange("p r f -> p (r f)"))
    nc.compile()
    return nc


_CACHE = {}


def kernel(x, edge_index, W1, b1, W2, b2):
    x = np.asarray(x, np.float32)
    if "k" not in _CACHE:
        dinv, meta, per_core = _preprocess(np.asarray(edge_index))
        ncx = _build(meta)
        _CACHE["k"] = (ncx, meta, per_core)
    ncx, meta, per_core = _CACHE["k"]

    iota8 = np.broadcast_to(np.arange(128, dtype=np.float32)[None, None, :],
                            (128, TPC, 128)).copy()
    ident = np.eye(128, dtype=np.float32)
    b1t = np.tile(np.asarray(b1, np.float32), (128, 1))
    b2t = np.tile(np.asarray(b2, np.float32), (128, 1))

    import ml_dtypes
    bf = ml_dtypes.bfloat16

    def hilo(a):
        a = np.asarray(a, np.float32)
        hi = a.astype(bf)
        lo = (a - hi.astype(np.float32)).astype(bf)
        return hi, lo

    W1h, W1l = hilo(W1)
    W2h, W2l = hilo(W2)
    in_maps = []
    for c in range(NC):
        pc = per_core[c]
        lo = c * SHARD
        xT = np.ascontiguousarray(x[lo:lo + SHARD].T)
        xTh, xTl = hilo(xT)
        in_maps.append(dict(
            xTh=xTh, xTl=xTl, W1h=W1h, W1l=W1l, W2h=W2h, W2l=W2l,
            b1t=b1t, b2t=b2t,
            dinv_col=pc["dinv_col"], iota8=iota8, ident=ident,
            idx=pc["idx"], dstcol=pc["dstcol"],
        ))
    _CACHE["inmaps"] = in_maps
    res = run_bass_kernel_spmd(ncx, in_maps, list(range(NC)))
    outs = []
    for c in range(NC):
        o = res.results[c]["out"].reshape(128, NREG, F_OUT)
        o = o.transpose(1, 0, 2).reshape(PSHARD, F_OUT)[:SHARD]
        outs.append(o)
    return np.concatenate(outs, axis=0).astype(np.float32)


def timed_run():
    """Re-run the cached compiled program with NTFF tracing; returns ns."""
    if "k" not in _CACHE or "inmaps" not in _CACHE:
        return None
    ncx = _CACHE["k"][0]
    in_maps = _CACHE["inmaps"]
    import tempfile
    try:
        res = run_bass_kernel_spmd(ncx, in_maps, list(range(NC)), trace=True,
                                   tmpdir=tempfile.mkdtemp(prefix="gcn_trace_"))
        return res.exec_time_ns
    except Exception:
        return None


# revision 17
# speedup vs baseline: 1.5085x; 1.5085x over previous
"""2-layer GCN on 8 Trainium2 NeuronCores (Bass/Tile).

Node (dst) sharding across 8 cores. Host builds a static, core-uniform
gather/scatter schedule: edges bucketed by src%4 (table row = packed_row//4,
int16-safe), sorted by dst, tiled into 128-slot scatter tiles per
(bucket, dst-region-of-128); tile counts padded to the max across cores so
one SPMD program serves all cores. Per layer: chunked dma_gather (SWDGE,
64B payload / 256B stride) fetches y[src] edge-major; PE one-hot matmuls
segment-sum into persistent PSUM region banks; AllGather (DRAM bounce)
shares per-shard y tables between layers.

Math: z = x@W1; y1 = dinv*z; agg = sum_edges y[src] (per dst);
h = relu(dinv*agg1 + dinv*y1 + b1); y2 = dinv*h;
out = log_softmax((dinv*agg2 + dinv*y2)@W2 + b2).
"""
import sys

sys.path.insert(0, "/opt/trn_rl_repo")
import numpy as np

import concourse.bass as bass
import concourse.bacc as bacc
import concourse.tile as tile
import concourse.mybir as mybir
from concourse.bass_utils import run_bass_kernel_spmd

N_NODES = 100000
F_IN, F_HID, F_OUT = 128, 16, 20
NC = 8
SHARD = N_NODES // NC           # 12500
NREG = (SHARD + 127) // 128     # 98
PSHARD = NREG * 128             # 12544 (padded shard rows in table)
NTAB = NC * PSHARD              # 100352 table rows
NBUCKET = 4                     # src row % 4
CHUNK = 1024
TPC = CHUNK // 128              # tiles per chunk = 8
NQUEUES = 4

f32 = mybir.dt.float32
f32r = mybir.dt.float32r
bf16 = mybir.dt.bfloat16
i32 = mybir.dt.int32
i16 = mybir.dt.int16


def _row_of(node):
    return (node // SHARD) * PSHARD + (node % SHARD)


def _preprocess(edge_index):
    src = edge_index[0].astype(np.int64)
    dst = edge_index[1].astype(np.int64)
    deg = np.bincount(dst, minlength=N_NODES).astype(np.float64) + 1.0
    dinv_all = (1.0 / np.sqrt(deg)).astype(np.float32)
    srow = _row_of(src)

    per_core_ed = []
    counts = np.zeros((NC, NBUCKET, NREG), np.int64)
    for c in range(NC):
        lo = c * SHARD
        m = (dst >= lo) & (dst < lo + SHARD)
        s, d = srow[m], dst[m] - lo
        b = s % 4
        order = np.lexsort((d, b))
        s, d, b = s[order], d[order], b[order]
        reg = d // 128
        np.add.at(counts[c], (b, reg), 1)
        per_core_ed.append((s, d, b, reg))

    # uniform tile counts per (bucket, region)
    tcnt = np.maximum(np.ceil(counts / 128.0).astype(np.int64).max(axis=0), 1)
    ntiles = int(tcnt.sum())

    # static schedule (same for all cores): per tile -> (bucket, region);
    # pad each bucket to a multiple of TPC with dummy (b, 0) tiles so no
    # chunk crosses a bucket boundary.
    sched = []
    for b in range(NBUCKET):
        blist = []
        for r in range(NREG):
            blist.extend([(b, r)] * int(tcnt[b, r]))
        while len(blist) % TPC:
            blist.append((b, 0))
        sched.extend(blist)
    ntiles_pad = len(sched)
    nchunks = ntiles_pad // TPC

    first, last = {}, {}
    for t, (b, r) in enumerate(sched):
        if r not in first:
            first[r] = t
        last[r] = t

    chunk_bucket = [sched[ci * TPC][0] for ci in range(nchunks)]
    # tiles within one chunk must share the bucket (gather src sub-table).
    for ci in range(nchunks):
        bs = {sched[ci * TPC + k][0] for k in range(TPC)}
        assert len(bs) == 1, f"chunk {ci} mixes buckets {bs}"

    meta = dict(ntiles=ntiles_pad, nchunks=nchunks, sched=sched,
                first=first, last=last, chunk_bucket=chunk_bucket)

    # per-core data arrays following the schedule
    per_core = []
    for c in range(NC):
        s, d, b, reg = per_core_ed[c]
        idx = np.zeros((ntiles_pad, 128), np.int16)
        dcol = np.full((ntiles_pad, 128), -1.0, np.float32)
        # edges of (b, r) are contiguous in (s, d) by construction
        starts = {}
        pos = 0
        for bb in range(NBUCKET):
            for r in range(NREG):
                starts[(bb, r)] = pos
                pos += int(counts[c, bb, r])
        fill = {}
        for t, (bb, r) in enumerate(sched):
            fill.setdefault((bb, r), []).append(t)
        for (bb, r), tlist in fill.items():
            p0 = starts.get((bb, r))
            if p0 is None:
                continue
            cnt = int(counts[c, bb, r]) if (bb, r) in starts else 0
            for k, t in enumerate(tlist):
                a, z_ = p0 + k * 128, p0 + min((k + 1) * 128, cnt)
                n = z_ - a
                if n <= 0:
                    continue
                idx[t, :n] = (s[a:z_] // 4).astype(np.int16)
                dcol[t, :n] = (d[a:z_] - r * 128).astype(np.float32)
        # wrap idx into [128, ntiles_pad*8] int16 (16-partition wrap, x8)
        iw = idx.reshape(ntiles_pad * 8, 16).T     # [16, ntiles*8]
        iw = np.tile(iw, (8, 1)).astype(np.int16)  # [128, ntiles*8]
        dct = dcol.T.copy()                        # [128, ntiles_pad]

        lo = c * SHARD
        dv = dinv_all[lo:lo + SHARD]
        dv = np.concatenate([dv, np.zeros(PSHARD - SHARD, np.float32)])
        dv = dv.reshape(NREG, 128).T.copy()        # [128, NREG]
        per_core.append(dict(idx=iw, dstcol=dct, dinv_col=dv))
    return dinv_all, meta, per_core


def _dma_gather_raw(gpsimd, out_ap, in_ap, idxs_ap, num_idxs, elem_size,
                    elem_step, queue_num=0):
    stride_bytes = elem_step * mybir.dt.size(in_ap.dtype)
    assert stride_bytes % 256 == 0
    _in_ap = gpsimd.lower_ap_dma(in_ap, for_custom_bir_dma=True)
    _idxs_ap = gpsimd.lower_ap(idxs_ap)
    _out_ap = gpsimd.lower_ap(out_ap)
    return gpsimd.add_instruction(
        mybir.InstDMAGatherAnt(
            name=gpsimd.bass.get_next_instruction_name(),
            ins=[*_in_ap, _idxs_ap,
                 gpsimd.lower_val_access(gpsimd.to_reg(num_idxs))],
            outs=[_out_ap],
            transpose=False, num_idxs=num_idxs, elem_size=elem_size,
            stride_bytes_256=stride_bytes // 256, gen_mode=0,
            single_packet=True, queue_num=queue_num,
            sbuf_tokens_per_rank=0, sbuf_free_dim_per_rank=0,
            sbuf_free_dim_pad_per_rank=0, sbuf_byte_offset=0,
        ))


def _build(meta):
    nchunks = meta["nchunks"]
    ntiles = meta["ntiles"]
    sched = meta["sched"]
    first, last = meta["first"], meta["last"]
    chunk_bucket = meta["chunk_bucket"]

    nc = bacc.Bacc("TRN2", target_bir_lowering=False, debug=False,
                   num_devices=NC, num_swdge_queues=NQUEUES)

    xh_d = nc.dram_tensor("xTh", [F_IN, SHARD], bf16, kind="ExternalInput")
    xl_d = nc.dram_tensor("xTl", [F_IN, SHARD], bf16, kind="ExternalInput")
    w1h_d = nc.dram_tensor("W1h", [F_IN, F_HID], bf16, kind="ExternalInput")
    w1l_d = nc.dram_tensor("W1l", [F_IN, F_HID], bf16, kind="ExternalInput")
    w2h_d = nc.dram_tensor("W2h", [F_HID, F_OUT], bf16, kind="ExternalInput")
    w2l_d = nc.dram_tensor("W2l", [F_HID, F_OUT], bf16, kind="ExternalInput")
    b1_d = nc.dram_tensor("b1t", [128, F_HID], f32, kind="ExternalInput")
    b2_d = nc.dram_tensor("b2t", [128, F_OUT], f32, kind="ExternalInput")
    dinv_d = nc.dram_tensor("dinv_col", [128, NREG], f32, kind="ExternalInput")
    iota_d = nc.dram_tensor("iota8", [128, TPC, 128], f32, kind="ExternalInput")
    ident_d = nc.dram_tensor("ident", [128, 128], f32, kind="ExternalInput")
    idx_d = nc.dram_tensor("idx", [128, ntiles * 8], i16, kind="ExternalInput")
    dc_d = nc.dram_tensor("dstcol", [128, ntiles], f32, kind="ExternalInput")
    out_d = nc.dram_tensor("out", [128, NREG * F_OUT], f32,
                           kind="ExternalOutput")


    ybounce = nc.dram_tensor("ybounce", [PSHARD, F_HID], f32)
    ytab1 = nc.dram_tensor("ytab1", [NTAB, F_HID], f32, addr_space="Shared")
    hbounce = nc.dram_tensor("hbounce", [PSHARD, F_HID], f32)
    ytab2 = nc.dram_tensor("ytab2", [NTAB, F_HID], f32, addr_space="Shared")

    with tile.TileContext(nc) as tc:
        with tc.tile_pool(name="const", bufs=1) as cpool, \
             tc.tile_pool(name="work", bufs=3) as wpool, \
             tc.tile_pool(name="gath", bufs=6) as gpool, \
             tc.tile_pool(name="big", bufs=1) as apool, \
             tc.tile_pool(name="ps", bufs=2, space="PSUM") as pspool, \
             tc.tile_pool(name="psacc", bufs=1, space="PSUM") as papool:

            w1h = cpool.tile([F_IN, F_HID], bf16)
            w1l = cpool.tile([F_IN, F_HID], bf16)
            w2h = cpool.tile([F_HID, F_OUT], bf16)
            w2l = cpool.tile([F_HID, F_OUT], bf16)
            b1t = cpool.tile([128, F_HID], f32)
            b2t = cpool.tile([128, F_OUT], f32)
            dinvc = cpool.tile([128, NREG], f32)
            iota8 = cpool.tile([128, TPC, 128], f32)
            ident = cpool.tile([128, 128], f32)
            for t_, d_ in ((w1h, w1h_d), (w1l, w1l_d), (w2h, w2h_d),
                           (w2l, w2l_d), (b1t, b1_d), (b2t, b2_d),
                           (dinvc, dinv_d), (iota8, iota_d), (ident, ident_d)):
                nc.sync.dma_start(out=t_[:], in_=d_[:])

            # phase A: z = x @ W1, dst-major
            xTh = apool.tile([F_IN, SHARD], bf16)
            xTl = apool.tile([F_IN, SHARD], bf16)
            nc.sync.dma_start(out=xTh[:], in_=xh_d[:])
            nc.sync.dma_start(out=xTl[:], in_=xl_d[:])
            z = apool.tile([128, NREG, F_HID], f32)
            for r in range(NREG):
                n0, n1 = r * 128, min(r * 128 + 128, SHARD)
                zp = pspool.tile([128, 128], f32, tag="pp")
                nc.tensor.matmul(out=zp[:n1 - n0, :F_HID],
                                 lhsT=xTh[:, n0:n1], rhs=w1h[:],
                                 start=True, stop=False, skip_group_check=True)
                nc.tensor.matmul(out=zp[:n1 - n0, :F_HID],
                                 lhsT=xTh[:, n0:n1], rhs=w1l[:],
                                 start=False, stop=False, skip_group_check=True)
                nc.tensor.matmul(out=zp[:n1 - n0, :F_HID],
                                 lhsT=xTl[:, n0:n1], rhs=w1h[:],
                                 start=False, stop=True, skip_group_check=True)
                if n1 - n0 < 128:
                    nc.vector.memset(z[:, r, :], 0.0)
                nc.vector.tensor_copy(out=z[:n1 - n0, r, :],
                                      in_=zp[:n1 - n0, :F_HID])

            dinv_b = dinvc[:].rearrange("p (r o) -> p r o", o=1)

            def aggregate(ytab, acc):
                nbank = (NREG + 31) // 32
                banks = []
                for i in range(nbank):
                    bk = papool.tile([128, 32 * F_HID], f32, tag=f"bk{i}",
                                     name=f"bank{i}")
                    banks.append(bk)
                for ci in range(nchunks):
                    b = chunk_bucket[ci]
                    gt = gpool.tile([128, TPC, F_HID], f32, tag="g")
                    ix = gpool.tile([128, CHUNK // 16], i16, tag="ix")
                    dc = gpool.tile([128, TPC], f32, tag="dc")
                    sel = gpool.tile([128, TPC, 128], f32, tag="sel")
                    nc.sync.dma_start(
                        out=ix[:],
                        in_=idx_d[:, ci * (CHUNK // 16):(ci + 1) * (CHUNK // 16)])
                    nc.sync.dma_start(out=dc[:],
                                      in_=dc_d[:, ci * TPC:(ci + 1) * TPC])
                    in_ap = bass.AP(ytab, b * F_HID,
                                    [[4 * F_HID, NTAB // 4], [1, F_HID]])
                    _dma_gather_raw(nc.gpsimd, gt[:], in_ap, ix[:], CHUNK,
                                    F_HID, 4 * F_HID, queue_num=ci % NQUEUES)
                    nc.vector.tensor_tensor(
                        out=sel[:],
                        in0=dc[:].rearrange("p (t o) -> p t o", o=1).to_broadcast(
                            [128, TPC, 128]),
                        in1=iota8[:], op=mybir.AluOpType.is_equal)
                    for tt in range(TPC):
                        t = ci * TPC + tt
                        _, reg = sched[t]
                        bank = banks[reg // 32]
                        off = (reg % 32) * F_HID
                        nc.tensor.matmul(
                            out=bank[:, off:off + F_HID],
                            lhsT=sel[:, tt, :], rhs=gt[:, tt, :],
                            start=(first[reg] == t), stop=(last[reg] == t),
                            skip_group_check=True)
                for bi, bank in enumerate(banks):
                    r0, r1 = bi * 32, min(bi * 32 + 32, NREG)
                    nc.vector.tensor_copy(
                        out=acc[:, r0:r1, :].rearrange("p r f -> p (r f)"),
                        in_=bank[:, :(r1 - r0) * F_HID])

            acc = apool.tile([128, NREG, F_HID], f32)

            # layer 1
            y1 = apool.tile([128, NREG, F_HID], f32)
            nc.vector.tensor_tensor(out=y1[:], in0=z[:],
                                    in1=dinv_b.to_broadcast([128, NREG, F_HID]),
                                    op=mybir.AluOpType.mult)
            nc.sync.dma_start(
                out=ybounce[:].rearrange("(r p) f -> p r f", p=128), in_=y1[:])
            nc.gpsimd.collective_compute(
                "AllGather", mybir.AluOpType.bypass,
                replica_groups=[list(range(NC))],
                ins=[ybounce[:]], outs=[ytab1[:]])
            aggregate(ytab1, acc)
            h = apool.tile([128, NREG, F_HID], f32)
            ta = apool.tile([128, NREG, F_HID], f32)
            tb = apool.tile([128, NREG, F_HID], f32)
            nc.vector.tensor_tensor(out=ta[:], in0=acc[:], in1=y1[:],
                                    op=mybir.AluOpType.add)
            nc.vector.tensor_tensor(out=tb[:], in0=ta[:],
                                    in1=dinv_b.to_broadcast([128, NREG, F_HID]),
                                    op=mybir.AluOpType.mult)
            nc.vector.tensor_tensor(
                out=ta[:], in0=tb[:],
                in1=b1t[:].rearrange("p (o f) -> p o f", o=1).to_broadcast(
                    [128, NREG, F_HID]),
                op=mybir.AluOpType.add)
            nc.scalar.activation(out=h[:], in_=ta[:],
                                 func=mybir.ActivationFunctionType.Relu)

            # layer 2
            y2 = y1
            nc.vector.tensor_tensor(out=y2[:], in0=h[:],
                                    in1=dinv_b.to_broadcast([128, NREG, F_HID]),
                                    op=mybir.AluOpType.mult)
            nc.sync.dma_start(
                out=hbounce[:].rearrange("(r p) f -> p r f", p=128), in_=y2[:])
            nc.gpsimd.collective_compute(
                "AllGather", mybir.AluOpType.bypass,
                replica_groups=[list(range(NC))],
                ins=[hbounce[:]], outs=[ytab2[:]])
            acc2 = apool.tile([128, NREG, F_HID], f32)
            aggregate(ytab2, acc2)
            tc2 = apool.tile([128, NREG, F_HID], f32)
            nc.vector.tensor_tensor(out=tc2[:], in0=acc2[:], in1=y2[:],
                                    op=mybir.AluOpType.add)
            nc.vector.tensor_tensor(out=acc2[:], in0=tc2[:],
                                    in1=dinv_b.to_broadcast([128, NREG, F_HID]),
                                    op=mybir.AluOpType.mult)

            out_sb = apool.tile([128, NREG, F_OUT], f32)
            for r in range(NREG):
                fp = pspool.tile([128, 128], f32, tag="pp")
                nc.tensor.transpose(out=fp[:F_HID, :], in_=acc2[:, r, :],
                                    identity=ident[:])
                fts = wpool.tile([F_HID, 128], f32, tag="ft")
                nc.vector.tensor_copy(out=fts[:], in_=fp[:F_HID, :])
                fh = wpool.tile([F_HID, 128], bf16, tag="fh")
                fhf = wpool.tile([F_HID, 128], f32, tag="fhf")
                fl = wpool.tile([F_HID, 128], bf16, tag="fl")
                nc.vector.tensor_copy(out=fh[:], in_=fts[:])
                nc.vector.tensor_copy(out=fhf[:], in_=fh[:])
                nc.vector.tensor_tensor(out=fl[:], in0=fts[:], in1=fhf[:],
                                        op=mybir.AluOpType.subtract)
                op = pspool.tile([128, 128], f32, tag="pp2")
                nc.tensor.matmul(out=op[:, :F_OUT], lhsT=fh[:], rhs=w2h[:],
                                 start=True, stop=False, skip_group_check=True)
                nc.tensor.matmul(out=op[:, :F_OUT], lhsT=fh[:], rhs=w2l[:],
                                 start=False, stop=False, skip_group_check=True)
                nc.tensor.matmul(out=op[:, :F_OUT], lhsT=fl[:], rhs=w2h[:],
                                 start=False, stop=True, skip_group_check=True)
                o = wpool.tile([128, F_OUT], f32, tag="o")
                nc.vector.tensor_tensor(out=o[:], in0=op[:, :F_OUT], in1=b2t[:],
                                        op=mybir.AluOpType.add)
                mx = wpool.tile([128, 1], f32, tag="mx")
                nc.vector.tensor_reduce(out=mx[:], in_=o[:],
                                        axis=mybir.AxisListType.X,
                                        op=mybir.AluOpType.max)
                nmx = wpool.tile([128, 1], f32, tag="nmx")
                nc.vector.tensor_scalar_mul(out=nmx[:], in0=mx[:], scalar1=-1.0)
                ex = wpool.tile([128, F_OUT], f32, tag="ex")
                sm = wpool.tile([128, 1], f32, tag="sm")
                nc.scalar.activation(out=ex[:], in_=o[:],
                                     func=mybir.ActivationFunctionType.Exp,
                                     bias=nmx[:, 0:1], accum_out=sm[:])
                ls = wpool.tile([128, 1], f32, tag="ls")
                nc.scalar.activation(out=ls[:], in_=sm[:],
                                     func=mybir.ActivationFunctionType.Ln)
                tot = wpool.tile([128, 1], f32, tag="tot")
                nc.vector.tensor_tensor(out=tot[:], in0=mx[:], in1=ls[:],
                                        op=mybir.AluOpType.add)
                nc.vector.tensor_tensor(
                    out=out_sb[:, r, :], in0=o[:],
                    in1=tot[:].to_broadcast([128, F_OUT]),
                    op=mybir.AluOpType.subtract)
            nc.sync.dma_start(out=out_d[:],
                              in_=out_sb[:].rearrange("p r f -> p (r f)"))
    nc.compile()
    return nc


_CACHE = {}


def kernel(x, edge_index, W1, b1, W2, b2):
    x = np.asarray(x, np.float32)
    if "k" not in _CACHE:
        dinv, meta, per_core = _preprocess(np.asarray(edge_index))
        ncx = _build(meta)
        _CACHE["k"] = (ncx, meta, per_core)
    ncx, meta, per_core = _CACHE["k"]

    iota8 = np.broadcast_to(np.arange(128, dtype=np.float32)[None, None, :],
                            (128, TPC, 128)).copy()
    ident = np.eye(128, dtype=np.float32)
    b1t = np.tile(np.asarray(b1, np.float32), (128, 1))
    b2t = np.tile(np.asarray(b2, np.float32), (128, 1))

    import ml_dtypes
    bf = ml_dtypes.bfloat16

    def hilo(a):
        a = np.asarray(a, np.float32)
        hi = a.astype(bf)
        lo = (a - hi.astype(np.float32)).astype(bf)
        return hi, lo

    W1h, W1l = hilo(W1)
    W2h, W2l = hilo(W2)
    in_maps = []
    for c in range(NC):
        pc = per_core[c]
        lo = c * SHARD
        xT = np.ascontiguousarray(x[lo:lo + SHARD].T)
        xTh, xTl = hilo(xT)
        in_maps.append(dict(
            xTh=xTh, xTl=xTl, W1h=W1h, W1l=W1l, W2h=W2h, W2l=W2l,
            b1t=b1t, b2t=b2t,
            dinv_col=pc["dinv_col"], iota8=iota8, ident=ident,
            idx=pc["idx"], dstcol=pc["dstcol"],
        ))
    _CACHE["inmaps"] = in_maps
    res = run_bass_kernel_spmd(ncx, in_maps, list(range(NC)))
    outs = []
    for c in range(NC):
        o = res.results[c]["out"].reshape(128, NREG, F_OUT)
        o = o.transpose(1, 0, 2).reshape(PSHARD, F_OUT)[:SHARD]
        outs.append(o)
    return np.concatenate(outs, axis=0).astype(np.float32)


def timed_run():
    """Re-run the cached compiled program with NTFF tracing; returns ns."""
    if "k" not in _CACHE or "inmaps" not in _CACHE:
        return None
    ncx = _CACHE["k"][0]
    in_maps = _CACHE["inmaps"]
    import tempfile
    try:
        res = run_bass_kernel_spmd(ncx, in_maps, list(range(NC)), trace=True,
                                   tmpdir=tempfile.mkdtemp(prefix="gcn_trace_"))
        return res.exec_time_ns
    except Exception:
        return None


# revision 18
# speedup vs baseline: 1.5199x; 1.0075x over previous
"""2-layer GCN on 8 Trainium2 NeuronCores (Bass/Tile).

Node (dst) sharding across 8 cores. Host builds a static, core-uniform
gather/scatter schedule: edges bucketed by src%4 (table row = packed_row//4,
int16-safe), sorted by dst, tiled into 128-slot scatter tiles per
(bucket, dst-region-of-128); tile counts padded to the max across cores so
one SPMD program serves all cores. Per layer: chunked dma_gather (SWDGE,
64B payload / 256B stride) fetches y[src] edge-major; PE one-hot matmuls
segment-sum into persistent PSUM region banks; AllGather (DRAM bounce)
shares per-shard y tables between layers.

Math: z = x@W1; y1 = dinv*z; agg = sum_edges y[src] (per dst);
h = relu(dinv*agg1 + dinv*y1 + b1); y2 = dinv*h;
out = log_softmax((dinv*agg2 + dinv*y2)@W2 + b2).
"""
import sys

sys.path.insert(0, "/opt/trn_rl_repo")
import numpy as np

import concourse.bass as bass
import concourse.bacc as bacc
import concourse.tile as tile
import concourse.mybir as mybir
from concourse.bass_utils import run_bass_kernel_spmd

N_NODES = 100000
F_IN, F_HID, F_OUT = 128, 16, 20
NC = 8
SHARD = N_NODES // NC           # 12500
NREG = (SHARD + 127) // 128     # 98
PSHARD = NREG * 128             # 12544 (padded shard rows in table)
NTAB = NC * PSHARD              # 100352 table rows
NBUCKET = 4                     # src row % 4
CHUNK = 1024
TPC = CHUNK // 128              # tiles per chunk = 8
NQUEUES = 4

f32 = mybir.dt.float32
f32r = mybir.dt.float32r
bf16 = mybir.dt.bfloat16
i32 = mybir.dt.int32
i16 = mybir.dt.int16


def _row_of(node):
    return (node // SHARD) * PSHARD + (node % SHARD)


def _preprocess(edge_index):
    src = edge_index[0].astype(np.int64)
    dst = edge_index[1].astype(np.int64)
    deg = np.bincount(dst, minlength=N_NODES).astype(np.float64) + 1.0
    dinv_all = (1.0 / np.sqrt(deg)).astype(np.float32)
    srow = _row_of(src)

    per_core_ed = []
    counts = np.zeros((NC, NBUCKET, NREG), np.int64)
    for c in range(NC):
        lo = c * SHARD
        m = (dst >= lo) & (dst < lo + SHARD)
        s, d = srow[m], dst[m] - lo
        b = s % 4
        order = np.lexsort((d, b))
        s, d, b = s[order], d[order], b[order]
        reg = d // 128
        np.add.at(counts[c], (b, reg), 1)
        per_core_ed.append((s, d, b, reg))

    # uniform tile counts per (bucket, region)
    tcnt = np.maximum(np.ceil(counts / 128.0).astype(np.int64).max(axis=0), 1)
    ntiles = int(tcnt.sum())

    # static schedule (same for all cores): per tile -> (bucket, region);
    # pad each bucket to a multiple of TPC with dummy (b, 0) tiles so no
    # chunk crosses a bucket boundary.
    sched = []
    for b in range(NBUCKET):
        blist = []
        for r in range(NREG):
            blist.extend([(b, r)] * int(tcnt[b, r]))
        while len(blist) % TPC:
            blist.append((b, 0))
        sched.extend(blist)
    ntiles_pad = len(sched)
    nchunks = ntiles_pad // TPC

    first, last = {}, {}
    for t, (b, r) in enumerate(sched):
        if r not in first:
            first[r] = t
        last[r] = t

    chunk_bucket = [sched[ci * TPC][0] for ci in range(nchunks)]
    # tiles within one chunk must share the bucket (gather src sub-table).
    for ci in range(nchunks):
        bs = {sched[ci * TPC + k][0] for k in range(TPC)}
        assert len(bs) == 1, f"chunk {ci} mixes buckets {bs}"

    meta = dict(ntiles=ntiles_pad, nchunks=nchunks, sched=sched,
                first=first, last=last, chunk_bucket=chunk_bucket)

    # per-core data arrays following the schedule
    per_core = []
    for c in range(NC):
        s, d, b, reg = per_core_ed[c]
        idx = np.zeros((ntiles_pad, 128), np.int16)
        dcol = np.full((ntiles_pad, 128), -1.0, np.float32)
        # edges of (b, r) are contiguous in (s, d) by construction
        starts = {}
        pos = 0
        for bb in range(NBUCKET):
            for r in range(NREG):
                starts[(bb, r)] = pos
                pos += int(counts[c, bb, r])
        fill = {}
        for t, (bb, r) in enumerate(sched):
            fill.setdefault((bb, r), []).append(t)
        for (bb, r), tlist in fill.items():
            p0 = starts.get((bb, r))
            if p0 is None:
                continue
            cnt = int(counts[c, bb, r]) if (bb, r) in starts else 0
            for k, t in enumerate(tlist):
                a, z_ = p0 + k * 128, p0 + min((k + 1) * 128, cnt)
                n = z_ - a
                if n <= 0:
                    continue
                idx[t, :n] = (s[a:z_] // 4).astype(np.int16)
                dcol[t, :n] = (d[a:z_] - r * 128).astype(np.float32)
        # wrap idx into [128, ntiles_pad*8] int16 (16-partition wrap, x8)
        iw = idx.reshape(ntiles_pad * 8, 16).T     # [16, ntiles*8]
        iw = np.tile(iw, (8, 1)).astype(np.int16)  # [128, ntiles*8]
        dct = dcol.T.copy()                        # [128, ntiles_pad]

        lo = c * SHARD
        dv = dinv_all[lo:lo + SHARD]
        dv = np.concatenate([dv, np.zeros(PSHARD - SHARD, np.float32)])
        dv = dv.reshape(NREG, 128).T.copy()        # [128, NREG]
        per_core.append(dict(idx=iw, dstcol=dct, dinv_col=dv))
    return dinv_all, meta, per_core


def _dma_gather_raw(gpsimd, out_ap, in_ap, idxs_ap, num_idxs, elem_size,
                    elem_step, queue_num=0):
    stride_bytes = elem_step * mybir.dt.size(in_ap.dtype)
    assert stride_bytes % 256 == 0
    _in_ap = gpsimd.lower_ap_dma(in_ap, for_custom_bir_dma=True)
    _idxs_ap = gpsimd.lower_ap(idxs_ap)
    _out_ap = gpsimd.lower_ap(out_ap)
    return gpsimd.add_instruction(
        mybir.InstDMAGatherAnt(
            name=gpsimd.bass.get_next_instruction_name(),
            ins=[*_in_ap, _idxs_ap,
                 gpsimd.lower_val_access(gpsimd.to_reg(num_idxs))],
            outs=[_out_ap],
            transpose=False, num_idxs=num_idxs, elem_size=elem_size,
            stride_bytes_256=stride_bytes // 256, gen_mode=0,
            single_packet=True, queue_num=queue_num,
            sbuf_tokens_per_rank=0, sbuf_free_dim_per_rank=0,
            sbuf_free_dim_pad_per_rank=0, sbuf_byte_offset=0,
        ))


def _build(meta):
    nchunks = meta["nchunks"]
    ntiles = meta["ntiles"]
    sched = meta["sched"]
    first, last = meta["first"], meta["last"]
    chunk_bucket = meta["chunk_bucket"]

    nc = bacc.Bacc("TRN2", target_bir_lowering=False, debug=False,
                   num_devices=NC, num_swdge_queues=NQUEUES)

    xh_d = nc.dram_tensor("xTh", [F_IN, SHARD], bf16, kind="ExternalInput")
    xl_d = nc.dram_tensor("xTl", [F_IN, SHARD], bf16, kind="ExternalInput")
    w1h_d = nc.dram_tensor("W1h", [F_IN, F_HID], bf16, kind="ExternalInput")
    w1l_d = nc.dram_tensor("W1l", [F_IN, F_HID], bf16, kind="ExternalInput")
    w2h_d = nc.dram_tensor("W2h", [F_HID, F_OUT], bf16, kind="ExternalInput")
    w2l_d = nc.dram_tensor("W2l", [F_HID, F_OUT], bf16, kind="ExternalInput")
    b1_d = nc.dram_tensor("b1t", [128, F_HID], f32, kind="ExternalInput")
    b2_d = nc.dram_tensor("b2t", [128, F_OUT], f32, kind="ExternalInput")
    dinv_d = nc.dram_tensor("dinv_col", [128, NREG], f32, kind="ExternalInput")
    iota_d = nc.dram_tensor("iota8", [128, TPC, 128], f32, kind="ExternalInput")
    ident_d = nc.dram_tensor("ident", [128, 128], f32, kind="ExternalInput")
    idx_d = nc.dram_tensor("idx", [128, ntiles * 8], i16, kind="ExternalInput")
    dc_d = nc.dram_tensor("dstcol", [128, ntiles], f32, kind="ExternalInput")
    out_d = nc.dram_tensor("out", [128, NREG * F_OUT], f32,
                           kind="ExternalOutput")


    ybounce = nc.dram_tensor("ybounce", [PSHARD, F_HID], f32)
    ytab1 = nc.dram_tensor("ytab1", [NTAB, F_HID], f32, addr_space="Shared")
    hbounce = nc.dram_tensor("hbounce", [PSHARD, F_HID], f32)
    ytab2 = nc.dram_tensor("ytab2", [NTAB, F_HID], f32, addr_space="Shared")

    with tile.TileContext(nc) as tc:
        with tc.tile_pool(name="const", bufs=1) as cpool, \
             tc.tile_pool(name="work", bufs=3) as wpool, \
             tc.tile_pool(name="gath", bufs=12) as gpool, \
             tc.tile_pool(name="big", bufs=1) as apool, \
             tc.tile_pool(name="ps", bufs=2, space="PSUM") as pspool, \
             tc.tile_pool(name="psacc", bufs=1, space="PSUM") as papool:

            w1h = cpool.tile([F_IN, F_HID], bf16)
            w1l = cpool.tile([F_IN, F_HID], bf16)
            w2h = cpool.tile([F_HID, F_OUT], bf16)
            w2l = cpool.tile([F_HID, F_OUT], bf16)
            b1t = cpool.tile([128, F_HID], f32)
            b2t = cpool.tile([128, F_OUT], f32)
            dinvc = cpool.tile([128, NREG], f32)
            iota8 = cpool.tile([128, TPC, 128], f32)
            ident = cpool.tile([128, 128], f32)
            for t_, d_ in ((w1h, w1h_d), (w1l, w1l_d), (w2h, w2h_d),
                           (w2l, w2l_d), (b1t, b1_d), (b2t, b2_d),
                           (dinvc, dinv_d), (iota8, iota_d), (ident, ident_d)):
                nc.sync.dma_start(out=t_[:], in_=d_[:])

            # phase A: z = x @ W1, dst-major
            xTh = apool.tile([F_IN, SHARD], bf16)
            xTl = apool.tile([F_IN, SHARD], bf16)
            nc.sync.dma_start(out=xTh[:], in_=xh_d[:])
            nc.sync.dma_start(out=xTl[:], in_=xl_d[:])
            z = apool.tile([128, NREG, F_HID], f32)
            for r in range(NREG):
                n0, n1 = r * 128, min(r * 128 + 128, SHARD)
                zp = pspool.tile([128, 128], f32, tag="pp")
                nc.tensor.matmul(out=zp[:n1 - n0, :F_HID],
                                 lhsT=xTh[:, n0:n1], rhs=w1h[:],
                                 start=True, stop=False, skip_group_check=True)
                nc.tensor.matmul(out=zp[:n1 - n0, :F_HID],
                                 lhsT=xTh[:, n0:n1], rhs=w1l[:],
                                 start=False, stop=False, skip_group_check=True)
                nc.tensor.matmul(out=zp[:n1 - n0, :F_HID],
                                 lhsT=xTl[:, n0:n1], rhs=w1h[:],
                                 start=False, stop=True, skip_group_check=True)
                if n1 - n0 < 128:
                    nc.vector.memset(z[:, r, :], 0.0)
                nc.vector.tensor_copy(out=z[:n1 - n0, r, :],
                                      in_=zp[:n1 - n0, :F_HID])

            dinv_b = dinvc[:].rearrange("p (r o) -> p r o", o=1)

            def aggregate(ytab, acc):
                nbank = (NREG + 31) // 32
                banks = []
                for i in range(nbank):
                    bk = papool.tile([128, 32 * F_HID], f32, tag=f"bk{i}",
                                     name=f"bank{i}")
                    banks.append(bk)
                for ci in range(nchunks):
                    b = chunk_bucket[ci]
                    gt = gpool.tile([128, TPC, F_HID], f32, tag="g")
                    ix = gpool.tile([128, CHUNK // 16], i16, tag="ix")
                    dc = gpool.tile([128, TPC], f32, tag="dc")
                    sel = gpool.tile([128, TPC, 128], f32, tag="sel")
                    nc.sync.dma_start(
                        out=ix[:],
                        in_=idx_d[:, ci * (CHUNK // 16):(ci + 1) * (CHUNK // 16)])
                    nc.sync.dma_start(out=dc[:],
                                      in_=dc_d[:, ci * TPC:(ci + 1) * TPC])
                    in_ap = bass.AP(ytab, b * F_HID,
                                    [[4 * F_HID, NTAB // 4], [1, F_HID]])
                    _dma_gather_raw(nc.gpsimd, gt[:], in_ap, ix[:], CHUNK,
                                    F_HID, 4 * F_HID, queue_num=ci % NQUEUES)
                    nc.vector.tensor_tensor(
                        out=sel[:],
                        in0=dc[:].rearrange("p (t o) -> p t o", o=1).to_broadcast(
                            [128, TPC, 128]),
                        in1=iota8[:], op=mybir.AluOpType.is_equal)
                    for tt in range(TPC):
                        t = ci * TPC + tt
                        _, reg = sched[t]
                        bank = banks[reg // 32]
                        off = (reg % 32) * F_HID
                        nc.tensor.matmul(
                            out=bank[:, off:off + F_HID],
                            lhsT=sel[:, tt, :], rhs=gt[:, tt, :],
                            start=(first[reg] == t), stop=(last[reg] == t),
                            skip_group_check=True)
                for bi, bank in enumerate(banks):
                    r0, r1 = bi * 32, min(bi * 32 + 32, NREG)
                    nc.vector.tensor_copy(
                        out=acc[:, r0:r1, :].rearrange("p r f -> p (r f)"),
                        in_=bank[:, :(r1 - r0) * F_HID])

            acc = apool.tile([128, NREG, F_HID], f32)

            # layer 1
            y1 = apool.tile([128, NREG, F_HID], f32)
            nc.vector.tensor_tensor(out=y1[:], in0=z[:],
                                    in1=dinv_b.to_broadcast([128, NREG, F_HID]),
                                    op=mybir.AluOpType.mult)
            nc.sync.dma_start(
                out=ybounce[:].rearrange("(r p) f -> p r f", p=128), in_=y1[:])
            nc.gpsimd.collective_compute(
                "AllGather", mybir.AluOpType.bypass,
                replica_groups=[list(range(NC))],
                ins=[ybounce[:]], outs=[ytab1[:]])
            aggregate(ytab1, acc)
            h = apool.tile([128, NREG, F_HID], f32)
            ta = apool.tile([128, NREG, F_HID], f32)
            tb = apool.tile([128, NREG, F_HID], f32)
            nc.vector.tensor_tensor(out=ta[:], in0=acc[:], in1=y1[:],
                                    op=mybir.AluOpType.add)
            nc.vector.tensor_tensor(out=tb[:], in0=ta[:],
                                    in1=dinv_b.to_broadcast([128, NREG, F_HID]),
                                    op=mybir.AluOpType.mult)
            nc.vector.tensor_tensor(
                out=ta[:], in0=tb[:],
                in1=b1t[:].rearrange("p (o f) -> p o f", o=1).to_broadcast(
                    [128, NREG, F_HID]),
                op=mybir.AluOpType.add)
            nc.scalar.activation(out=h[:], in_=ta[:],
                                 func=mybir.ActivationFunctionType.Relu)

            # layer 2
            y2 = y1
            nc.vector.tensor_tensor(out=y2[:], in0=h[:],
                                    in1=dinv_b.to_broadcast([128, NREG, F_HID]),
                                    op=mybir.AluOpType.mult)
            nc.sync.dma_start(
                out=hbounce[:].rearrange("(r p) f -> p r f", p=128), in_=y2[:])
            nc.gpsimd.collective_compute(
                "AllGather", mybir.AluOpType.bypass,
                replica_groups=[list(range(NC))],
                ins=[hbounce[:]], outs=[ytab2[:]])
            acc2 = apool.tile([128, NREG, F_HID], f32)
            aggregate(ytab2, acc2)
            tc2 = apool.tile([128, NREG, F_HID], f32)
            nc.vector.tensor_tensor(out=tc2[:], in0=acc2[:], in1=y2[:],
                                    op=mybir.AluOpType.add)
            nc.vector.tensor_tensor(out=acc2[:], in0=tc2[:],
                                    in1=dinv_b.to_broadcast([128, NREG, F_HID]),
                                    op=mybir.AluOpType.mult)

            out_sb = apool.tile([128, NREG, F_OUT], f32)
            for r in range(NREG):
                fp = pspool.tile([128, 128], f32, tag="pp")
                nc.tensor.transpose(out=fp[:F_HID, :], in_=acc2[:, r, :],
                                    identity=ident[:])
                fts = wpool.tile([F_HID, 128], f32, tag="ft")
                nc.vector.tensor_copy(out=fts[:], in_=fp[:F_HID, :])
                fh = wpool.tile([F_HID, 128], bf16, tag="fh")
                fhf = wpool.tile([F_HID, 128], f32, tag="fhf")
                fl = wpool.tile([F_HID, 128], bf16, tag="fl")
                nc.vector.tensor_copy(out=fh[:], in_=fts[:])
                nc.vector.tensor_copy(out=fhf[:], in_=fh[:])
                nc.vector.tensor_tensor(out=fl[:], in0=fts[:], in1=fhf[:],
                                        op=mybir.AluOpType.subtract)
                op = pspool.tile([128, 128], f32, tag="pp2")
                nc.tensor.matmul(out=op[:, :F_OUT], lhsT=fh[:], rhs=w2h[:],
                                 start=True, stop=False, skip_group_check=True)
                nc.tensor.matmul(out=op[:, :F_OUT], lhsT=fh[:], rhs=w2l[:],
                                 start=False, stop=False, skip_group_check=True)
                nc.tensor.matmul(out=op[:, :F_OUT], lhsT=fl[:], rhs=w2h[:],
                                 start=False, stop=True, skip_group_check=True)
                o = wpool.tile([128, F_OUT], f32, tag="o")
                nc.vector.tensor_tensor(out=o[:], in0=op[:, :F_OUT], in1=b2t[:],
                                        op=mybir.AluOpType.add)
                mx = wpool.tile([128, 1], f32, tag="mx")
                nc.vector.tensor_reduce(out=mx[:], in_=o[:],
                                        axis=mybir.AxisListType.X,
                                        op=mybir.AluOpType.max)
                nmx = wpool.tile([128, 1], f32, tag="nmx")
                nc.vector.tensor_scalar_mul(out=nmx[:], in0=mx[:], scalar1=-1.0)
                ex = wpool.tile([128, F_OUT], f32, tag="ex")
                sm = wpool.tile([128, 1], f32, tag="sm")
                nc.scalar.activation(out=ex[:], in_=o[:],
                                     func=mybir.ActivationFunctionType.Exp,
                                     bias=nmx[:, 0:1], accum_out=sm[:])
                ls = wpool.tile([128, 1], f32, tag="ls")
                nc.scalar.activation(out=ls[:], in_=sm[:],
                                     func=mybir.ActivationFunctionType.Ln)
                tot = wpool.tile([128, 1], f32, tag="tot")
                nc.vector.tensor_tensor(out=tot[:], in0=mx[:], in1=ls[:],
                                        op=mybir.AluOpType.add)
                nc.vector.tensor_tensor(
                    out=out_sb[:, r, :], in0=o[:],
                    in1=tot[:].to_broadcast([128, F_OUT]),
                    op=mybir.AluOpType.subtract)
            nc.sync.dma_start(out=out_d[:],
                              in_=out_sb[:].rearrange("p r f -> p (r f)"))
    nc.compile()
    return nc


_CACHE = {}


def kernel(x, edge_index, W1, b1, W2, b2):
    x = np.asarray(x, np.float32)
    if "k" not in _CACHE:
        dinv, meta, per_core = _preprocess(np.asarray(edge_index))
        ncx = _build(meta)
        _CACHE["k"] = (ncx, meta, per_core)
    ncx, meta, per_core = _CACHE["k"]

    iota8 = np.broadcast_to(np.arange(128, dtype=np.float32)[None, None, :],
                            (128, TPC, 128)).copy()
    ident = np.eye(128, dtype=np.float32)
    b1t = np.tile(np.asarray(b1, np.float32), (128, 1))
    b2t = np.tile(np.asarray(b2, np.float32), (128, 1))

    import ml_dtypes
    bf = ml_dtypes.bfloat16

    def hilo(a):
        a = np.asarray(a, np.float32)
        hi = a.astype(bf)
        lo = (a - hi.astype(np.float32)).astype(bf)
        return hi, lo

    W1h, W1l = hilo(W1)
    W2h, W2l = hilo(W2)
    in_maps = []
    for c in range(NC):
        pc = per_core[c]
        lo = c * SHARD
        xT = np.ascontiguousarray(x[lo:lo + SHARD].T)
        xTh, xTl = hilo(xT)
        in_maps.append(dict(
            xTh=xTh, xTl=xTl, W1h=W1h, W1l=W1l, W2h=W2h, W2l=W2l,
            b1t=b1t, b2t=b2t,
            dinv_col=pc["dinv_col"], iota8=iota8, ident=ident,
            idx=pc["idx"], dstcol=pc["dstcol"],
        ))
    _CACHE["inmaps"] = in_maps
    res = run_bass_kernel_spmd(ncx, in_maps, list(range(NC)))
    outs = []
    for c in range(NC):
        o = res.results[c]["out"].reshape(128, NREG, F_OUT)
        o = o.transpose(1, 0, 2).reshape(PSHARD, F_OUT)[:SHARD]
        outs.append(o)
    return np.concatenate(outs, axis=0).astype(np.float32)


def timed_run():
    """Re-run the cached compiled program with NTFF tracing; returns ns."""
    if "k" not in _CACHE or "inmaps" not in _CACHE:
        return None
    ncx = _CACHE["k"][0]
    in_maps = _CACHE["inmaps"]
    import tempfile
    try:
        res = run_bass_kernel_spmd(ncx, in_maps, list(range(NC)), trace=True,
                                   tmpdir=tempfile.mkdtemp(prefix="gcn_trace_"))
        return res.exec_time_ns
    except Exception:
        return None


# revision 19
# speedup vs baseline: 1.5212x; 1.0008x over previous
"""2-layer GCN on 8 Trainium2 NeuronCores (Bass/Tile).

Node (dst) sharding across 8 cores. Host builds a static, core-uniform
gather/scatter schedule: edges bucketed by src%4 (table row = packed_row//4,
int16-safe), sorted by dst, tiled into 128-slot scatter tiles per
(bucket, dst-region-of-128); tile counts padded to the max across cores so
one SPMD program serves all cores. Per layer: chunked dma_gather (SWDGE,
64B payload / 256B stride) fetches y[src] edge-major; PE one-hot matmuls
segment-sum into persistent PSUM region banks; AllGather (DRAM bounce)
shares per-shard y tables between layers.

Math: z = x@W1; y1 = dinv*z; agg = sum_edges y[src] (per dst);
h = relu(dinv*agg1 + dinv*y1 + b1); y2 = dinv*h;
out = log_softmax((dinv*agg2 + dinv*y2)@W2 + b2).
"""
import sys

sys.path.insert(0, "/opt/trn_rl_repo")
import numpy as np

import concourse.bass as bass
import concourse.bacc as bacc
import concourse.tile as tile
import concourse.mybir as mybir
from concourse.bass_utils import run_bass_kernel_spmd

N_NODES = 100000
F_IN, F_HID, F_OUT = 128, 16, 20
NC = 8
SHARD = N_NODES // NC           # 12500
NREG = (SHARD + 127) // 128     # 98
PSHARD = NREG * 128             # 12544 (padded shard rows in table)
NTAB = NC * PSHARD              # 100352 table rows
NBUCKET = 4                     # src row % 4
CHUNK = 1024
TPC = CHUNK // 128              # tiles per chunk = 8
NQUEUES = 4

f32 = mybir.dt.float32
f32r = mybir.dt.float32r
bf16 = mybir.dt.bfloat16
i32 = mybir.dt.int32
i16 = mybir.dt.int16


def _row_of(node):
    return (node // SHARD) * PSHARD + (node % SHARD)


def _preprocess(edge_index):
    src = edge_index[0].astype(np.int64)
    dst = edge_index[1].astype(np.int64)
    deg = np.bincount(dst, minlength=N_NODES).astype(np.float64) + 1.0
    dinv_all = (1.0 / np.sqrt(deg)).astype(np.float32)
    srow = _row_of(src)

    per_core_ed = []
    counts = np.zeros((NC, NBUCKET, NREG), np.int64)
    for c in range(NC):
        lo = c * SHARD
        m = (dst >= lo) & (dst < lo + SHARD)
        s, d = srow[m], dst[m] - lo
        b = s % 4
        order = np.lexsort((d, b))
        s, d, b = s[order], d[order], b[order]
        reg = d // 128
        np.add.at(counts[c], (b, reg), 1)
        per_core_ed.append((s, d, b, reg))

    # uniform tile counts per (bucket, region)
    tcnt = np.maximum(np.ceil(counts / 128.0).astype(np.int64).max(axis=0), 1)
    ntiles = int(tcnt.sum())

    # static schedule (same for all cores): per tile -> (bucket, region);
    # pad each bucket to a multiple of TPC with dummy (b, 0) tiles so no
    # chunk crosses a bucket boundary.
    sched = []
    for b in range(NBUCKET):
        blist = []
        for r in range(NREG):
            blist.extend([(b, r)] * int(tcnt[b, r]))
        while len(blist) % TPC:
            blist.append((b, 0))
        sched.extend(blist)
    ntiles_pad = len(sched)
    nchunks = ntiles_pad // TPC

    first, last = {}, {}
    for t, (b, r) in enumerate(sched):
        if r not in first:
            first[r] = t
        last[r] = t

    chunk_bucket = [sched[ci * TPC][0] for ci in range(nchunks)]
    # tiles within one chunk must share the bucket (gather src sub-table).
    for ci in range(nchunks):
        bs = {sched[ci * TPC + k][0] for k in range(TPC)}
        assert len(bs) == 1, f"chunk {ci} mixes buckets {bs}"

    meta = dict(ntiles=ntiles_pad, nchunks=nchunks, sched=sched,
                first=first, last=last, chunk_bucket=chunk_bucket)

    # per-core data arrays following the schedule
    per_core = []
    for c in range(NC):
        s, d, b, reg = per_core_ed[c]
        idx = np.zeros((ntiles_pad, 128), np.int16)
        dcol = np.full((ntiles_pad, 128), -1.0, np.float32)
        # edges of (b, r) are contiguous in (s, d) by construction
        starts = {}
        pos = 0
        for bb in range(NBUCKET):
            for r in range(NREG):
                starts[(bb, r)] = pos
                pos += int(counts[c, bb, r])
        fill = {}
        for t, (bb, r) in enumerate(sched):
            fill.setdefault((bb, r), []).append(t)
        for (bb, r), tlist in fill.items():
            p0 = starts.get((bb, r))
            if p0 is None:
                continue
            cnt = int(counts[c, bb, r]) if (bb, r) in starts else 0
            for k, t in enumerate(tlist):
                a, z_ = p0 + k * 128, p0 + min((k + 1) * 128, cnt)
                n = z_ - a
                if n <= 0:
                    continue
                idx[t, :n] = (s[a:z_] // 4).astype(np.int16)
                dcol[t, :n] = (d[a:z_] - r * 128).astype(np.float32)
        # wrap idx into [128, ntiles_pad*8] int16 (16-partition wrap, x8)
        iw = idx.reshape(ntiles_pad * 8, 16).T     # [16, ntiles*8]
        iw = np.tile(iw, (8, 1)).astype(np.int16)  # [128, ntiles*8]
        dct = dcol.T.copy()                        # [128, ntiles_pad]

        lo = c * SHARD
        dv = dinv_all[lo:lo + SHARD]
        dv = np.concatenate([dv, np.zeros(PSHARD - SHARD, np.float32)])
        dv = dv.reshape(NREG, 128).T.copy()        # [128, NREG]
        per_core.append(dict(idx=iw, dstcol=dct, dinv_col=dv))
    return dinv_all, meta, per_core


def _dma_gather_raw(gpsimd, out_ap, in_ap, idxs_ap, num_idxs, elem_size,
                    elem_step, queue_num=0):
    stride_bytes = elem_step * mybir.dt.size(in_ap.dtype)
    assert stride_bytes % 256 == 0
    _in_ap = gpsimd.lower_ap_dma(in_ap, for_custom_bir_dma=True)
    _idxs_ap = gpsimd.lower_ap(idxs_ap)
    _out_ap = gpsimd.lower_ap(out_ap)
    return gpsimd.add_instruction(
        mybir.InstDMAGatherAnt(
            name=gpsimd.bass.get_next_instruction_name(),
            ins=[*_in_ap, _idxs_ap,
                 gpsimd.lower_val_access(gpsimd.to_reg(num_idxs))],
            outs=[_out_ap],
            transpose=False, num_idxs=num_idxs, elem_size=elem_size,
            stride_bytes_256=stride_bytes // 256, gen_mode=0,
            single_packet=True, queue_num=queue_num,
            sbuf_tokens_per_rank=0, sbuf_free_dim_per_rank=0,
            sbuf_free_dim_pad_per_rank=0, sbuf_byte_offset=0,
        ))


def _build(meta):
    nchunks = meta["nchunks"]
    ntiles = meta["ntiles"]
    sched = meta["sched"]
    first, last = meta["first"], meta["last"]
    chunk_bucket = meta["chunk_bucket"]

    nc = bacc.Bacc("TRN2", target_bir_lowering=False, debug=False,
                   num_devices=NC, num_swdge_queues=NQUEUES)

    xh_d = nc.dram_tensor("xTh", [F_IN, SHARD], bf16, kind="ExternalInput")
    xl_d = nc.dram_tensor("xTl", [F_IN, SHARD], bf16, kind="ExternalInput")
    w1h_d = nc.dram_tensor("W1h", [F_IN, F_HID], bf16, kind="ExternalInput")
    w1l_d = nc.dram_tensor("W1l", [F_IN, F_HID], bf16, kind="ExternalInput")
    w2h_d = nc.dram_tensor("W2h", [F_HID, F_OUT], bf16, kind="ExternalInput")
    w2l_d = nc.dram_tensor("W2l", [F_HID, F_OUT], bf16, kind="ExternalInput")
    b1_d = nc.dram_tensor("b1t", [128, F_HID], f32, kind="ExternalInput")
    b2_d = nc.dram_tensor("b2t", [128, F_OUT], f32, kind="ExternalInput")
    dinv_d = nc.dram_tensor("dinv_col", [128, NREG], f32, kind="ExternalInput")
    iota_d = nc.dram_tensor("iota8", [128, TPC, 128], f32, kind="ExternalInput")
    ident_d = nc.dram_tensor("ident", [128, 128], f32, kind="ExternalInput")
    idx_d = nc.dram_tensor("idx", [128, ntiles * 8], i16, kind="ExternalInput")
    dc_d = nc.dram_tensor("dstcol", [128, ntiles], f32, kind="ExternalInput")
    out_d = nc.dram_tensor("out", [128, NREG * F_OUT], f32,
                           kind="ExternalOutput")


    ybounce = nc.dram_tensor("ybounce", [PSHARD, F_HID], f32)
    ytab1 = nc.dram_tensor("ytab1", [NTAB, F_HID], f32, addr_space="Shared")
    hbounce = nc.dram_tensor("hbounce", [PSHARD, F_HID], f32)
    ytab2 = nc.dram_tensor("ytab2", [NTAB, F_HID], f32, addr_space="Shared")

    with tile.TileContext(nc) as tc:
        with tc.tile_pool(name="const", bufs=1) as cpool, \
             tc.tile_pool(name="work", bufs=3) as wpool, \
             tc.tile_pool(name="gath", bufs=12) as gpool, \
             tc.tile_pool(name="big", bufs=1) as apool, \
             tc.tile_pool(name="ps", bufs=2, space="PSUM") as pspool, \
             tc.tile_pool(name="psacc", bufs=1, space="PSUM") as papool:

            w1h = cpool.tile([F_IN, F_HID], bf16)
            w1l = cpool.tile([F_IN, F_HID], bf16)
            w2h = cpool.tile([F_HID, F_OUT], bf16)
            w2l = cpool.tile([F_HID, F_OUT], bf16)
            b1t = cpool.tile([128, F_HID], f32)
            b2t = cpool.tile([128, F_OUT], f32)
            dinvc = cpool.tile([128, NREG], f32)
            iota8 = cpool.tile([128, TPC, 128], f32)
            ident = cpool.tile([128, 128], f32)
            for t_, d_ in ((w1h, w1h_d), (w1l, w1l_d), (w2h, w2h_d),
                           (w2l, w2l_d), (b1t, b1_d), (b2t, b2_d),
                           (dinvc, dinv_d), (iota8, iota_d), (ident, ident_d)):
                nc.sync.dma_start(out=t_[:], in_=d_[:])

            # phase A: z = x @ W1, dst-major
            xTh = apool.tile([F_IN, SHARD], bf16)
            xTl = apool.tile([F_IN, SHARD], bf16)
            nc.sync.dma_start(out=xTh[:], in_=xh_d[:])
            nc.sync.dma_start(out=xTl[:], in_=xl_d[:])
            z = apool.tile([128, NREG, F_HID], f32)
            for r in range(NREG):
                n0, n1 = r * 128, min(r * 128 + 128, SHARD)
                zp = pspool.tile([128, 128], f32, tag="pp")
                nc.tensor.matmul(out=zp[:n1 - n0, :F_HID],
                                 lhsT=xTh[:, n0:n1], rhs=w1h[:],
                                 start=True, stop=False, skip_group_check=True)
                nc.tensor.matmul(out=zp[:n1 - n0, :F_HID],
                                 lhsT=xTh[:, n0:n1], rhs=w1l[:],
                                 start=False, stop=False, skip_group_check=True)
                nc.tensor.matmul(out=zp[:n1 - n0, :F_HID],
                                 lhsT=xTl[:, n0:n1], rhs=w1h[:],
                                 start=False, stop=True, skip_group_check=True)
                if n1 - n0 < 128:
                    nc.vector.memset(z[:, r, :], 0.0)
                nc.vector.tensor_copy(out=z[:n1 - n0, r, :],
                                      in_=zp[:n1 - n0, :F_HID])

            dinv_b = dinvc[:].rearrange("p (r o) -> p r o", o=1)

            def aggregate(ytab, acc):
                nbank = (NREG + 31) // 32
                banks = []
                for i in range(nbank):
                    bk = papool.tile([128, 32 * F_HID], f32, tag=f"bk{i}",
                                     name=f"bank{i}")
                    banks.append(bk)
                GB = 8
                gix = None
                for ci in range(nchunks):
                    b = chunk_bucket[ci]
                    if ci % GB == 0:
                        gn = min(GB, nchunks - ci)
                        gix = gpool.tile([128, GB * (CHUNK // 16)], i16,
                                         tag="gix", name="gix")
                        gdc = gpool.tile([128, GB * TPC], f32, tag="gdc",
                                         name="gdc")
                        nc.scalar.dma_start(
                            out=gix[:, :gn * (CHUNK // 16)],
                            in_=idx_d[:, ci * (CHUNK // 16):
                                      (ci + gn) * (CHUNK // 16)])
                        nc.sync.dma_start(
                            out=gdc[:, :gn * TPC],
                            in_=dc_d[:, ci * TPC:(ci + gn) * TPC])
                    g = ci % GB
                    ix = gix[:, g * (CHUNK // 16):(g + 1) * (CHUNK // 16)]
                    dc = gdc[:, g * TPC:(g + 1) * TPC]
                    gt = gpool.tile([128, TPC, F_HID], f32, tag="g")
                    sel = gpool.tile([128, TPC, 128], f32, tag="sel")
                    in_ap = bass.AP(ytab, b * F_HID,
                                    [[4 * F_HID, NTAB // 4], [1, F_HID]])
                    _dma_gather_raw(nc.gpsimd, gt[:], in_ap, ix, CHUNK,
                                    F_HID, 4 * F_HID, queue_num=ci % NQUEUES)
                    nc.vector.tensor_tensor(
                        out=sel[:],
                        in0=dc.rearrange("p (t o) -> p t o", o=1).to_broadcast(
                            [128, TPC, 128]),
                        in1=iota8[:], op=mybir.AluOpType.is_equal)
                    for tt in range(TPC):
                        t = ci * TPC + tt
                        _, reg = sched[t]
                        bank = banks[reg // 32]
                        off = (reg % 32) * F_HID
                        nc.tensor.matmul(
                            out=bank[:, off:off + F_HID],
                            lhsT=sel[:, tt, :], rhs=gt[:, tt, :],
                            start=(first[reg] == t), stop=(last[reg] == t),
                            skip_group_check=True)
                for bi, bank in enumerate(banks):
                    r0, r1 = bi * 32, min(bi * 32 + 32, NREG)
                    nc.vector.tensor_copy(
                        out=acc[:, r0:r1, :].rearrange("p r f -> p (r f)"),
                        in_=bank[:, :(r1 - r0) * F_HID])

            acc = apool.tile([128, NREG, F_HID], f32)

            # layer 1
            y1 = apool.tile([128, NREG, F_HID], f32)
            nc.vector.tensor_tensor(out=y1[:], in0=z[:],
                                    in1=dinv_b.to_broadcast([128, NREG, F_HID]),
                                    op=mybir.AluOpType.mult)
            nc.sync.dma_start(
                out=ybounce[:].rearrange("(r p) f -> p r f", p=128), in_=y1[:])
            nc.gpsimd.collective_compute(
                "AllGather", mybir.AluOpType.bypass,
                replica_groups=[list(range(NC))],
                ins=[ybounce[:]], outs=[ytab1[:]])
            aggregate(ytab1, acc)
            h = apool.tile([128, NREG, F_HID], f32)
            ta = apool.tile([128, NREG, F_HID], f32)
            tb = apool.tile([128, NREG, F_HID], f32)
            nc.vector.tensor_tensor(out=ta[:], in0=acc[:], in1=y1[:],
                                    op=mybir.AluOpType.add)
            nc.vector.tensor_tensor(out=tb[:], in0=ta[:],
                                    in1=dinv_b.to_broadcast([128, NREG, F_HID]),
                                    op=mybir.AluOpType.mult)
            nc.vector.tensor_tensor(
                out=ta[:], in0=tb[:],
                in1=b1t[:].rearrange("p (o f) -> p o f", o=1).to_broadcast(
                    [128, NREG, F_HID]),
                op=mybir.AluOpType.add)
            nc.scalar.activation(out=h[:], in_=ta[:],
                                 func=mybir.ActivationFunctionType.Relu)

            # layer 2
            y2 = y1
            nc.vector.tensor_tensor(out=y2[:], in0=h[:],
                                    in1=dinv_b.to_broadcast([128, NREG, F_HID]),
                                    op=mybir.AluOpType.mult)
            nc.sync.dma_start(
                out=hbounce[:].rearrange("(r p) f -> p r f", p=128), in_=y2[:])
            nc.gpsimd.collective_compute(
                "AllGather", mybir.AluOpType.bypass,
                replica_groups=[list(range(NC))],
                ins=[hbounce[:]], outs=[ytab2[:]])
            acc2 = apool.tile([128, NREG, F_HID], f32)
            aggregate(ytab2, acc2)
            tc2 = apool.tile([128, NREG, F_HID], f32)
            nc.vector.tensor_tensor(out=tc2[:], in0=acc2[:], in1=y2[:],
                                    op=mybir.AluOpType.add)
            nc.vector.tensor_tensor(out=acc2[:], in0=tc2[:],
                                    in1=dinv_b.to_broadcast([128, NREG, F_HID]),
                                    op=mybir.AluOpType.mult)

            out_sb = apool.tile([128, NREG, F_OUT], f32)
            for r in range(NREG):
                fp = pspool.tile([128, 128], f32, tag="pp")
                nc.tensor.transpose(out=fp[:F_HID, :], in_=acc2[:, r, :],
                                    identity=ident[:])
                fts = wpool.tile([F_HID, 128], f32, tag="ft")
                nc.vector.tensor_copy(out=fts[:], in_=fp[:F_HID, :])
                fh = wpool.tile([F_HID, 128], bf16, tag="fh")
                fhf = wpool.tile([F_HID, 128], f32, tag="fhf")
                fl = wpool.tile([F_HID, 128], bf16, tag="fl")
                nc.vector.tensor_copy(out=fh[:], in_=fts[:])
                nc.vector.tensor_copy(out=fhf[:], in_=fh[:])
                nc.vector.tensor_tensor(out=fl[:], in0=fts[:], in1=fhf[:],
                                        op=mybir.AluOpType.subtract)
                op = pspool.tile([128, 128], f32, tag="pp2")
                nc.tensor.matmul(out=op[:, :F_OUT], lhsT=fh[:], rhs=w2h[:],
                                 start=True, stop=False, skip_group_check=True)
                nc.tensor.matmul(out=op[:, :F_OUT], lhsT=fh[:], rhs=w2l[:],
                                 start=False, stop=False, skip_group_check=True)
                nc.tensor.matmul(out=op[:, :F_OUT], lhsT=fl[:], rhs=w2h[:],
                                 start=False, stop=True, skip_group_check=True)
                o = wpool.tile([128, F_OUT], f32, tag="o")
                nc.vector.tensor_tensor(out=o[:], in0=op[:, :F_OUT], in1=b2t[:],
                                        op=mybir.AluOpType.add)
                mx = wpool.tile([128, 1], f32, tag="mx")
                nc.vector.tensor_reduce(out=mx[:], in_=o[:],
                                        axis=mybir.AxisListType.X,
                                        op=mybir.AluOpType.max)
                nmx = wpool.tile([128, 1], f32, tag="nmx")
                nc.vector.tensor_scalar_mul(out=nmx[:], in0=mx[:], scalar1=-1.0)
                ex = wpool.tile([128, F_OUT], f32, tag="ex")
                sm = wpool.tile([128, 1], f32, tag="sm")
                nc.scalar.activation(out=ex[:], in_=o[:],
                                     func=mybir.ActivationFunctionType.Exp,
                                     bias=nmx[:, 0:1], accum_out=sm[:])
                ls = wpool.tile([128, 1], f32, tag="ls")
                nc.scalar.activation(out=ls[:], in_=sm[:],
                                     func=mybir.ActivationFunctionType.Ln)
                tot = wpool.tile([128, 1], f32, tag="tot")
                nc.vector.tensor_tensor(out=tot[:], in0=mx[:], in1=ls[:],
                                        op=mybir.AluOpType.add)
                nc.vector.tensor_tensor(
                    out=out_sb[:, r, :], in0=o[:],
                    in1=tot[:].to_broadcast([128, F_OUT]),
                    op=mybir.AluOpType.subtract)
            nc.sync.dma_start(out=out_d[:],
                              in_=out_sb[:].rearrange("p r f -> p (r f)"))
    nc.compile()
    return nc


_CACHE = {}


def kernel(x, edge_index, W1, b1, W2, b2):
    x = np.asarray(x, np.float32)
    if "k" not in _CACHE:
        dinv, meta, per_core = _preprocess(np.asarray(edge_index))
        ncx = _build(meta)
        _CACHE["k"] = (ncx, meta, per_core)
    ncx, meta, per_core = _CACHE["k"]

    iota8 = np.broadcast_to(np.arange(128, dtype=np.float32)[None, None, :],
                            (128, TPC, 128)).copy()
    ident = np.eye(128, dtype=np.float32)
    b1t = np.tile(np.asarray(b1, np.float32), (128, 1))
    b2t = np.tile(np.asarray(b2, np.float32), (128, 1))

    import ml_dtypes
    bf = ml_dtypes.bfloat16

    def hilo(a):
        a = np.asarray(a, np.float32)
        hi = a.astype(bf)
        lo = (a - hi.astype(np.float32)).astype(bf)
        return hi, lo

    W1h, W1l = hilo(W1)
    W2h, W2l = hilo(W2)
    in_maps = []
    for c in range(NC):
        pc = per_core[c]
        lo = c * SHARD
        xT = np.ascontiguousarray(x[lo:lo + SHARD].T)
        xTh, xTl = hilo(xT)
        in_maps.append(dict(
            xTh=xTh, xTl=xTl, W1h=W1h, W1l=W1l, W2h=W2h, W2l=W2l,
            b1t=b1t, b2t=b2t,
            dinv_col=pc["dinv_col"], iota8=iota8, ident=ident,
            idx=pc["idx"], dstcol=pc["dstcol"],
        ))
    _CACHE["inmaps"] = in_maps
    res = run_bass_kernel_spmd(ncx, in_maps, list(range(NC)))
    outs = []
    for c in range(NC):
        o = res.results[c]["out"].reshape(128, NREG, F_OUT)
        o = o.transpose(1, 0, 2).reshape(PSHARD, F_OUT)[:SHARD]
        outs.append(o)
    return np.concatenate(outs, axis=0).astype(np.float32)


def timed_run():
    """Re-run the cached compiled program with NTFF tracing; returns ns."""
    if "k" not in _CACHE or "inmaps" not in _CACHE:
        return None
    ncx = _CACHE["k"][0]
    in_maps = _CACHE["inmaps"]
    import tempfile
    try:
        res = run_bass_kernel_spmd(ncx, in_maps, list(range(NC)), trace=True,
                                   tmpdir=tempfile.mkdtemp(prefix="gcn_trace_"))
        return res.exec_time_ns
    except Exception:
        return None


# revision 20
# speedup vs baseline: 1.6342x; 1.0743x over previous
"""2-layer GCN on 8 Trainium2 NeuronCores (Bass/Tile).

Node (dst) sharding across 8 cores. Host builds a static, core-uniform
gather/scatter schedule: edges bucketed by src%4 (table row = packed_row//4,
int16-safe), sorted by dst, tiled into 128-slot scatter tiles per
(bucket, dst-region-of-128); tile counts padded to the max across cores so
one SPMD program serves all cores. Per layer: chunked dma_gather (SWDGE,
64B payload / 256B stride) fetches y[src] edge-major; PE one-hot matmuls
segment-sum into persistent PSUM region banks; AllGather (DRAM bounce)
shares per-shard y tables between layers.

Math: z = x@W1; y1 = dinv*z; agg = sum_edges y[src] (per dst);
h = relu(dinv*agg1 + dinv*y1 + b1); y2 = dinv*h;
out = log_softmax((dinv*agg2 + dinv*y2)@W2 + b2).
"""
import sys

sys.path.insert(0, "/opt/trn_rl_repo")
import numpy as np

import concourse.bass as bass
import concourse.bacc as bacc
import concourse.tile as tile
import concourse.mybir as mybir
from concourse.bass_utils import run_bass_kernel_spmd

N_NODES = 100000
F_IN, F_HID, F_OUT = 128, 16, 20
NC = 8
SHARD = N_NODES // NC           # 12500
NREG = (SHARD + 127) // 128     # 98
PSHARD = NREG * 128             # 12544 (padded shard rows in table)
NTAB = NC * PSHARD              # 100352 table rows
NBUCKET = 4                     # src row % 4
CHUNK = 1024
TPC = CHUNK // 128              # tiles per chunk = 8
NQUEUES = 4

f32 = mybir.dt.float32
f32r = mybir.dt.float32r
bf16 = mybir.dt.bfloat16
i32 = mybir.dt.int32
i16 = mybir.dt.int16


def _row_of(node):
    return (node // SHARD) * PSHARD + (node % SHARD)


def _preprocess(edge_index):
    src = edge_index[0].astype(np.int64)
    dst = edge_index[1].astype(np.int64)
    deg = np.bincount(dst, minlength=N_NODES).astype(np.float64) + 1.0
    dinv_all = (1.0 / np.sqrt(deg)).astype(np.float32)
    srow = _row_of(src)

    per_core_ed = []
    counts = np.zeros((NC, NBUCKET, NREG), np.int64)
    for c in range(NC):
        lo = c * SHARD
        m = (dst >= lo) & (dst < lo + SHARD)
        s, d = srow[m], dst[m] - lo
        b = s % 4
        order = np.lexsort((d, b))
        s, d, b = s[order], d[order], b[order]
        reg = d // 128
        np.add.at(counts[c], (b, reg), 1)
        per_core_ed.append((s, d, b, reg))

    # uniform tile counts per (bucket, region)
    tcnt = np.maximum(np.ceil(counts / 128.0).astype(np.int64).max(axis=0), 1)
    ntiles = int(tcnt.sum())

    # static schedule (same for all cores): per tile -> (bucket, region);
    # pad each bucket to a multiple of TPC with dummy (b, 0) tiles so no
    # chunk crosses a bucket boundary.
    sched = []
    for b in range(NBUCKET):
        blist = []
        for r in range(NREG):
            blist.extend([(b, r)] * int(tcnt[b, r]))
        while len(blist) % TPC:
            blist.append((b, 0))
        sched.extend(blist)
    ntiles_pad = len(sched)
    nchunks = ntiles_pad // TPC

    first, last = {}, {}
    for t, (b, r) in enumerate(sched):
        if r not in first:
            first[r] = t
        last[r] = t

    chunk_bucket = [sched[ci * TPC][0] for ci in range(nchunks)]
    # tiles within one chunk must share the bucket (gather src sub-table).
    for ci in range(nchunks):
        bs = {sched[ci * TPC + k][0] for k in range(TPC)}
        assert len(bs) == 1, f"chunk {ci} mixes buckets {bs}"

    meta = dict(ntiles=ntiles_pad, nchunks=nchunks, sched=sched,
                first=first, last=last, chunk_bucket=chunk_bucket)

    # per-core data arrays following the schedule
    per_core = []
    for c in range(NC):
        s, d, b, reg = per_core_ed[c]
        idx = np.zeros((ntiles_pad, 128), np.int16)
        dcol = np.full((ntiles_pad, 128), -1.0, np.float32)
        # edges of (b, r) are contiguous in (s, d) by construction
        starts = {}
        pos = 0
        for bb in range(NBUCKET):
            for r in range(NREG):
                starts[(bb, r)] = pos
                pos += int(counts[c, bb, r])
        fill = {}
        for t, (bb, r) in enumerate(sched):
            fill.setdefault((bb, r), []).append(t)
        for (bb, r), tlist in fill.items():
            p0 = starts.get((bb, r))
            if p0 is None:
                continue
            cnt = int(counts[c, bb, r]) if (bb, r) in starts else 0
            for k, t in enumerate(tlist):
                a, z_ = p0 + k * 128, p0 + min((k + 1) * 128, cnt)
                n = z_ - a
                if n <= 0:
                    continue
                idx[t, :n] = (s[a:z_] // 4).astype(np.int16)
                dcol[t, :n] = (d[a:z_] - r * 128).astype(np.float32)
        # wrap idx into [128, ntiles_pad*8] int16 (16-partition wrap, x8)
        iw = idx.reshape(ntiles_pad * 8, 16).T     # [16, ntiles*8]
        iw = np.tile(iw, (8, 1)).astype(np.int16)  # [128, ntiles*8]
        dct = dcol.T.copy()                        # [128, ntiles_pad]

        lo = c * SHARD
        dv = dinv_all[lo:lo + SHARD]
        dv = np.concatenate([dv, np.zeros(PSHARD - SHARD, np.float32)])
        dv = dv.reshape(NREG, 128).T.copy()        # [128, NREG]
        per_core.append(dict(idx=iw, dstcol=dct, dinv_col=dv))
    return dinv_all, meta, per_core


def _dma_gather_raw(gpsimd, out_ap, in_ap, idxs_ap, num_idxs, elem_size,
                    elem_step, queue_num=0):
    stride_bytes = elem_step * mybir.dt.size(in_ap.dtype)
    assert stride_bytes % 256 == 0
    _in_ap = gpsimd.lower_ap_dma(in_ap, for_custom_bir_dma=True)
    _idxs_ap = gpsimd.lower_ap(idxs_ap)
    _out_ap = gpsimd.lower_ap(out_ap)
    return gpsimd.add_instruction(
        mybir.InstDMAGatherAnt(
            name=gpsimd.bass.get_next_instruction_name(),
            ins=[*_in_ap, _idxs_ap,
                 gpsimd.lower_val_access(gpsimd.to_reg(num_idxs))],
            outs=[_out_ap],
            transpose=False, num_idxs=num_idxs, elem_size=elem_size,
            stride_bytes_256=stride_bytes // 256, gen_mode=0,
            single_packet=True, queue_num=queue_num,
            sbuf_tokens_per_rank=0, sbuf_free_dim_per_rank=0,
            sbuf_free_dim_pad_per_rank=0, sbuf_byte_offset=0,
        ))


def _build(meta):
    nchunks = meta["nchunks"]
    ntiles = meta["ntiles"]
    sched = meta["sched"]
    first, last = meta["first"], meta["last"]
    chunk_bucket = meta["chunk_bucket"]

    nc = bacc.Bacc("TRN2", target_bir_lowering=False, debug=False,
                   num_devices=NC, num_swdge_queues=NQUEUES)

    xh_d = nc.dram_tensor("xTh", [F_IN, SHARD], bf16, kind="ExternalInput")
    xl_d = nc.dram_tensor("xTl", [F_IN, SHARD], bf16, kind="ExternalInput")
    w1h_d = nc.dram_tensor("W1h", [F_IN, F_HID], bf16, kind="ExternalInput")
    w1l_d = nc.dram_tensor("W1l", [F_IN, F_HID], bf16, kind="ExternalInput")
    w2h_d = nc.dram_tensor("W2h", [F_HID, F_OUT], bf16, kind="ExternalInput")
    w2l_d = nc.dram_tensor("W2l", [F_HID, F_OUT], bf16, kind="ExternalInput")
    b1_d = nc.dram_tensor("b1t", [128, F_HID], f32, kind="ExternalInput")
    b2_d = nc.dram_tensor("b2t", [128, F_OUT], f32, kind="ExternalInput")
    dinv_d = nc.dram_tensor("dinv_col", [128, NREG], f32, kind="ExternalInput")
    iota_d = nc.dram_tensor("iota8", [128, TPC, 128], f32, kind="ExternalInput")
    ident_d = nc.dram_tensor("ident", [128, 128], f32, kind="ExternalInput")
    idx_d = nc.dram_tensor("idx", [128, ntiles * 8], i16, kind="ExternalInput")
    dc_d = nc.dram_tensor("dstcol", [128, ntiles], f32, kind="ExternalInput")
    out_d = nc.dram_tensor("out", [128, NREG * F_OUT], f32,
                           kind="ExternalOutput")


    ybounce = nc.dram_tensor("ybounce", [PSHARD, F_HID], f32)
    ytab1 = nc.dram_tensor("ytab1", [NTAB, F_HID], f32, addr_space="Shared")
    hbounce = nc.dram_tensor("hbounce", [PSHARD, F_HID], f32)
    ytab2 = nc.dram_tensor("ytab2", [NTAB, F_HID], f32, addr_space="Shared")

    with tile.TileContext(nc) as tc:
        with tc.tile_pool(name="const", bufs=1) as cpool, \
             tc.tile_pool(name="work", bufs=3) as wpool, \
             tc.tile_pool(name="gath", bufs=12) as gpool, \
             tc.tile_pool(name="big", bufs=1) as apool, \
             tc.tile_pool(name="ps", bufs=2, space="PSUM") as pspool, \
             tc.tile_pool(name="psacc", bufs=1, space="PSUM") as papool:

            w1h = cpool.tile([F_IN, F_HID], bf16)
            w1l = cpool.tile([F_IN, F_HID], bf16)
            w2h = cpool.tile([F_HID, F_OUT], bf16)
            w2l = cpool.tile([F_HID, F_OUT], bf16)
            b1t = cpool.tile([128, F_HID], f32)
            b2t = cpool.tile([128, F_OUT], f32)
            dinvc = cpool.tile([128, NREG], f32)
            iota8 = cpool.tile([128, TPC, 128], f32)
            ident = cpool.tile([128, 128], f32)
            for t_, d_ in ((w1h, w1h_d), (w1l, w1l_d), (w2h, w2h_d),
                           (w2l, w2l_d), (b1t, b1_d), (b2t, b2_d),
                           (dinvc, dinv_d), (iota8, iota_d), (ident, ident_d)):
                nc.sync.dma_start(out=t_[:], in_=d_[:])

            # phase A: z = x @ W1, dst-major
            xTh = apool.tile([F_IN, SHARD], bf16)
            xTl = apool.tile([F_IN, SHARD], bf16)
            nc.sync.dma_start(out=xTh[:], in_=xh_d[:])
            nc.sync.dma_start(out=xTl[:], in_=xl_d[:])
            z = apool.tile([128, NREG, F_HID], f32)
            for r in range(NREG):
                n0, n1 = r * 128, min(r * 128 + 128, SHARD)
                zp = pspool.tile([128, 128], f32, tag="pp")
                nc.tensor.matmul(out=zp[:n1 - n0, :F_HID],
                                 lhsT=xTh[:, n0:n1], rhs=w1h[:],
                                 start=True, stop=False, skip_group_check=True)
                nc.tensor.matmul(out=zp[:n1 - n0, :F_HID],
                                 lhsT=xTh[:, n0:n1], rhs=w1l[:],
                                 start=False, stop=False, skip_group_check=True)
                nc.tensor.matmul(out=zp[:n1 - n0, :F_HID],
                                 lhsT=xTl[:, n0:n1], rhs=w1h[:],
                                 start=False, stop=True, skip_group_check=True)
                if n1 - n0 < 128:
                    nc.vector.memset(z[:, r, :], 0.0)
                nc.vector.tensor_copy(out=z[:n1 - n0, r, :],
                                      in_=zp[:n1 - n0, :F_HID])

            dinv_b = dinvc[:].rearrange("p (r o) -> p r o", o=1)

            def aggregate(ytab, acc):
                nbank = (NREG + 31) // 32
                banks = []
                for i in range(nbank):
                    bk = papool.tile([128, 32 * F_HID], f32, tag=f"bk{i}",
                                     name=f"bank{i}")
                    banks.append(bk)
                GB = 8
                gix = None
                for ci in range(nchunks):
                    b = chunk_bucket[ci]
                    if ci % GB == 0:
                        gn = min(GB, nchunks - ci)
                        gix = gpool.tile([128, GB * (CHUNK // 16)], i16,
                                         tag="gix", name="gix")
                        gdc = gpool.tile([128, GB * TPC], f32, tag="gdc",
                                         name="gdc")
                        nc.scalar.dma_start(
                            out=gix[:, :gn * (CHUNK // 16)],
                            in_=idx_d[:, ci * (CHUNK // 16):
                                      (ci + gn) * (CHUNK // 16)])
                        nc.sync.dma_start(
                            out=gdc[:, :gn * TPC],
                            in_=dc_d[:, ci * TPC:(ci + gn) * TPC])
                    g = ci % GB
                    ix = gix[:, g * (CHUNK // 16):(g + 1) * (CHUNK // 16)]
                    dc = gdc[:, g * TPC:(g + 1) * TPC]
                    gt = gpool.tile([128, TPC, F_HID], f32, tag="g")
                    sel = gpool.tile([128, TPC, 128], f32, tag="sel")
                    in_ap = bass.AP(ytab, b * F_HID,
                                    [[4 * F_HID, NTAB // 4], [1, F_HID]])
                    _dma_gather_raw(nc.gpsimd, gt[:], in_ap, ix, CHUNK,
                                    F_HID, 4 * F_HID, queue_num=ci % NQUEUES)
                    nc.vector.tensor_tensor(
                        out=sel[:],
                        in0=dc.rearrange("p (t o) -> p t o", o=1).to_broadcast(
                            [128, TPC, 128]),
                        in1=iota8[:], op=mybir.AluOpType.is_equal)
                    for tt in range(TPC):
                        t = ci * TPC + tt
                        _, reg = sched[t]
                        bank = banks[reg // 32]
                        off = (reg % 32) * F_HID
                        nc.tensor.matmul(
                            out=bank[:, off:off + F_HID],
                            lhsT=sel[:, tt, :], rhs=gt[:, tt, :],
                            start=(first[reg] == t), stop=(last[reg] == t),
                            skip_group_check=True)
                for bi, bank in enumerate(banks):
                    r0, r1 = bi * 32, min(bi * 32 + 32, NREG)
                    nc.vector.tensor_copy(
                        out=acc[:, r0:r1, :].rearrange("p r f -> p (r f)"),
                        in_=bank[:, :(r1 - r0) * F_HID])

            acc = apool.tile([128, NREG, F_HID], f32)

            # layer 1
            y1 = apool.tile([128, NREG, F_HID], f32)
            nc.vector.tensor_tensor(out=y1[:], in0=z[:],
                                    in1=dinv_b.to_broadcast([128, NREG, F_HID]),
                                    op=mybir.AluOpType.mult)
            nc.sync.dma_start(
                out=ybounce[:].rearrange("(r p) f -> p r f", p=128), in_=y1[:])
            nc.gpsimd.collective_compute(
                "AllGather", mybir.AluOpType.bypass,
                replica_groups=[list(range(NC))],
                ins=[ybounce[:]], outs=[ytab1[:]])
            aggregate(ytab1, acc)
            h = apool.tile([128, NREG, F_HID], f32)
            ta = apool.tile([128, NREG, F_HID], f32)
            tb = apool.tile([128, NREG, F_HID], f32)
            nc.vector.tensor_tensor(out=ta[:], in0=acc[:], in1=y1[:],
                                    op=mybir.AluOpType.add)
            nc.vector.tensor_tensor(out=tb[:], in0=ta[:],
                                    in1=dinv_b.to_broadcast([128, NREG, F_HID]),
                                    op=mybir.AluOpType.mult)
            nc.vector.tensor_tensor(
                out=ta[:], in0=tb[:],
                in1=b1t[:].rearrange("p (o f) -> p o f", o=1).to_broadcast(
                    [128, NREG, F_HID]),
                op=mybir.AluOpType.add)
            nc.scalar.activation(out=h[:], in_=ta[:],
                                 func=mybir.ActivationFunctionType.Relu)

            # layer 2
            y2 = y1
            nc.vector.tensor_tensor(out=y2[:], in0=h[:],
                                    in1=dinv_b.to_broadcast([128, NREG, F_HID]),
                                    op=mybir.AluOpType.mult)
            nc.sync.dma_start(
                out=hbounce[:].rearrange("(r p) f -> p r f", p=128), in_=y2[:])
            nc.gpsimd.collective_compute(
                "AllGather", mybir.AluOpType.bypass,
                replica_groups=[list(range(NC))],
                ins=[hbounce[:]], outs=[ytab2[:]])
            acc2 = apool.tile([128, NREG, F_HID], f32)
            aggregate(ytab2, acc2)
            tc2 = apool.tile([128, NREG, F_HID], f32)
            nc.vector.tensor_tensor(out=tc2[:], in0=acc2[:], in1=y2[:],
                                    op=mybir.AluOpType.add)
            nc.vector.tensor_tensor(out=acc2[:], in0=tc2[:],
                                    in1=dinv_b.to_broadcast([128, NREG, F_HID]),
                                    op=mybir.AluOpType.mult)

            out_sb = apool.tile([128, NREG, F_OUT], f32)
            for r in range(NREG):
                fp = pspool.tile([128, 128], f32, tag="pp")
                nc.tensor.transpose(out=fp[:F_HID, :], in_=acc2[:, r, :],
                                    identity=ident[:])
                fts = wpool.tile([F_HID, 128], f32, tag="ft")
                nc.vector.tensor_copy(out=fts[:], in_=fp[:F_HID, :])
                fh = wpool.tile([F_HID, 128], bf16, tag="fh")
                fhf = wpool.tile([F_HID, 128], f32, tag="fhf")
                fl = wpool.tile([F_HID, 128], bf16, tag="fl")
                nc.vector.tensor_copy(out=fh[:], in_=fts[:])
                nc.vector.tensor_copy(out=fhf[:], in_=fh[:])
                nc.vector.tensor_tensor(out=fl[:], in0=fts[:], in1=fhf[:],
                                        op=mybir.AluOpType.subtract)
                op = pspool.tile([128, 128], f32, tag="pp2")
                nc.tensor.matmul(out=op[:, :F_OUT], lhsT=fh[:], rhs=w2h[:],
                                 start=True, stop=False, skip_group_check=True)
                nc.tensor.matmul(out=op[:, :F_OUT], lhsT=fh[:], rhs=w2l[:],
                                 start=False, stop=False, skip_group_check=True)
                nc.tensor.matmul(out=op[:, :F_OUT], lhsT=fl[:], rhs=w2h[:],
                                 start=False, stop=True, skip_group_check=True)
                nc.vector.tensor_tensor(out=out_sb[:, r, :],
                                        in0=op[:, :F_OUT], in1=b2t[:],
                                        op=mybir.AluOpType.add)
            # batched log_softmax over the class axis
            mx = apool.tile([128, NREG, 1], f32)
            nc.vector.tensor_reduce(out=mx[:], in_=out_sb[:],
                                    axis=mybir.AxisListType.X,
                                    op=mybir.AluOpType.max)
            nc.vector.tensor_tensor(
                out=out_sb[:], in0=out_sb[:],
                in1=mx[:].to_broadcast([128, NREG, F_OUT]),
                op=mybir.AluOpType.subtract)
            exs = apool.tile([128, NREG, F_OUT], f32)
            nc.scalar.activation(out=exs[:], in_=out_sb[:],
                                 func=mybir.ActivationFunctionType.Exp)
            sm = apool.tile([128, NREG, 1], f32)
            nc.vector.tensor_reduce(out=sm[:], in_=exs[:],
                                    axis=mybir.AxisListType.X,
                                    op=mybir.AluOpType.add)
            ls = apool.tile([128, NREG, 1], f32)
            nc.scalar.activation(out=ls[:], in_=sm[:],
                                 func=mybir.ActivationFunctionType.Ln)
            nc.vector.tensor_tensor(
                out=out_sb[:], in0=out_sb[:],
                in1=ls[:].to_broadcast([128, NREG, F_OUT]),
                op=mybir.AluOpType.subtract)
            nc.sync.dma_start(out=out_d[:],
                              in_=out_sb[:].rearrange("p r f -> p (r f)"))
    nc.compile()
    return nc


_CACHE = {}


def kernel(x, edge_index, W1, b1, W2, b2):
    x = np.asarray(x, np.float32)
    if "k" not in _CACHE:
        dinv, meta, per_core = _preprocess(np.asarray(edge_index))
        ncx = _build(meta)
        _CACHE["k"] = (ncx, meta, per_core)
    ncx, meta, per_core = _CACHE["k"]

    iota8 = np.broadcast_to(np.arange(128, dtype=np.float32)[None, None, :],
                            (128, TPC, 128)).copy()
    ident = np.eye(128, dtype=np.float32)
    b1t = np.tile(np.asarray(b1, np.float32), (128, 1))
    b2t = np.tile(np.asarray(b2, np.float32), (128, 1))

    import ml_dtypes
    bf = ml_dtypes.bfloat16

    def hilo(a):
        a = np.asarray(a, np.float32)
        hi = a.astype(bf)
        lo = (a - hi.astype(np.float32)).astype(bf)
        return hi, lo

    W1h, W1l = hilo(W1)
    W2h, W2l = hilo(W2)
    in_maps = []
    for c in range(NC):
        pc = per_core[c]
        lo = c * SHARD
        xT = np.ascontiguousarray(x[lo:lo + SHARD].T)
        xTh, xTl = hilo(xT)
        in_maps.append(dict(
            xTh=xTh, xTl=xTl, W1h=W1h, W1l=W1l, W2h=W2h, W2l=W2l,
            b1t=b1t, b2t=b2t,
            dinv_col=pc["dinv_col"], iota8=iota8, ident=ident,
            idx=pc["idx"], dstcol=pc["dstcol"],
        ))
    _CACHE["inmaps"] = in_maps
    res = run_bass_kernel_spmd(ncx, in_maps, list(range(NC)))
    outs = []
    for c in range(NC):
        o = res.results[c]["out"].reshape(128, NREG, F_OUT)
        o = o.transpose(1, 0, 2).reshape(PSHARD, F_OUT)[:SHARD]
        outs.append(o)
    return np.concatenate(outs, axis=0).astype(np.float32)


def timed_run():
    """Re-run the cached compiled program with NTFF tracing; returns ns."""
    if "k" not in _CACHE or "inmaps" not in _CACHE:
        return None
    ncx = _CACHE["k"][0]
    in_maps = _CACHE["inmaps"]
    import tempfile
    try:
        res = run_bass_kernel_spmd(ncx, in_maps, list(range(NC)), trace=True,
                                   tmpdir=tempfile.mkdtemp(prefix="gcn_trace_"))
        return res.exec_time_ns
    except Exception:
        return None
